# revision 1
# baseline (speedup 1.0000x reference)
"""Two-branch attention kernel for Trainium2 (8 NeuronCores, batch-parallel).

out1 = proj(softmax(q k^T / 8) v),  out2 = proj(softmax(q k2^T / 8) v2)
with q,k,v from x and k2,v2 from x2 (q shared across branches).

Sharding: batch dim (8) -> one batch element per core. No collectives.

Layout strategy (per core, transpose-free attention):
  host passes x^T, x2^T, W_qkv^T, W_proj^T, broadcast bias.
  QKV:  qT,kT [dim,tok] via W-stationary matmuls; v [tok,dim] via
        x-stationary matmuls, written into a ones-augmented buffer.
        k2T/v2 spilled to DRAM to bound SBUF.
  Attn: S^T tile = kT(stationary) @ qT(moving); exp on ScalarE (scale=1/8,
        no max subtraction -- scores are ~N(0,1), exp range is safe) writes
        P^T straight to SBUF; AV accumulates o_aug^T = [v|1]^T @ P^T giving
        both o^T and the softmax denominators r; normalize o^T by 1/r with a
        partition-broadcast multiply.
  Proj: o^T tiles stationary, stream W_proj^T, add bias, DMA out.
"""
import sys
for _p in ('/opt/trn_rl_repo',):
    if _p not in sys.path:
        sys.path.insert(0, _p)

import numpy as np

# ----------------------------------------------------------------------------
# knobs
MODE = 'f32r+bf16p'  # informational only
OT64 = False         # True: o^T stored [64,12,1024] (no partition-offset
                     # writes, proj uses 12 K=64 tiles); False: [128,6,1024]
BCAST_DMA = False     # broadcast 1/r via DMA (else gpsimd partition_broadcast)

EXP_COMBINED = True
ACT_EVICT = True
STRUCT = 2           # 0: separate S psums + AV after exp + no proj interleave
                     # 1: shared S tile + AV interleaved
                     # 2: + proj thunks interleaved

B, N, D, H, HD = 8, 1024, 768, 12, 64
SCALE = HD ** -0.5
NDT = D // 128       # 6 dim tiles
NQT = N // 128       # 8 token tiles
P = 128

# ----------------------------------------------------------------------------
# workaround: walrus rejects >2 sem waits on one instruction; TileContext's
# tail drain carries one wait per active logical proc. Split them across
# single-wait SP nops and emit a bare drain.
def _install_tilefix():
    import bass_rust
    import concourse.tile as tile

    def _drain_and_barrier_split(self, tick_clock, wait_clock):
        gc = tick_clock.global_clock
        ticks = [gc[i] for i in range(27)]
        for i, t in enumerate(ticks):
            if t > 0:
                vc = bass_rust.VectorClock(
                    [t if j == i else 0 for j in range(len(ticks))])
                nop = self.nc.sync.nop()
                wait_clock.add_sem_waits(
                    nop.ins, bass_rust.ScopedClock({None: vc}))
        self.nc.sync.drain()
        self.nc.all_engine_barrier()
        assert self.sems is not None
        popped = self.nc._tile_sem_poison_stack.pop()
        assert popped is self._sem_poison
        self.nc.clear_and_free_semaphores(list(self.sems.allocated().values()))
        self.nc.all_engine_barrier()

    tile.TileContext._drain_and_barrier = _drain_and_barrier_split


def _split_multiwaits(nc, max_waits=1):
    """walrus codegen rejects instructions carrying more than `max_waits`
    sync waits; hoist the extras onto same-engine nops placed just before."""
    import bass_rust
    import concourse.mybir as mybir
    cnt = 0
    for bb in nc.main_func.blocks:
        insts = bb.instructions
        i = 0
        while i < len(insts):
            ins = insts[i]
            si = getattr(ins, 'sync_info', None)
            if si is not None and si.on_wait and len(si.on_wait) > max_waits:
                waits = list(si.on_wait)
                extras, keep = waits[:-max_waits], waits[-max_waits:]
                for w in extras:
                    nop = mybir.InstNoOp(name=f"I-swx{cnt}", ins=[], outs=[])
                    cnt += 1
                    nop.engine = ins.engine
                    nop.sync_info = bass_rust.SyncInfo(on_wait=[w],
                                                       on_update=[])
                    insts.insert(i, nop)
                    i += 1
                ins.sync_info = bass_rust.SyncInfo(
                    on_wait=keep, on_update=list(si.on_update))
            i += 1
    return cnt


_built = None


def _build():
    """Build the SPMD bass program once. Returns (nc, n_split_waits)."""
    global _built
    if _built is not None:
        return _built
    _install_tilefix()
    from contextlib import ExitStack
    import concourse.bass as bass
    import concourse.tile as tile
    from concourse import mybir

    dt = mybir.dt
    ddt = dt.float32r          # matmul dtype for QKV / QK / proj operands
    pdt = dt.bfloat16          # attention probabilities P^T and V storage

    nc = bass.Bass("TRN2", target_bir_lowering=False, debug=False,
                   num_devices=8)

    # DRAM I/O (per core)
    xt_d = nc.dram_tensor("xt", [D, N], ddt, kind="ExternalInput")
    x2t_d = nc.dram_tensor("x2t", [D, N], ddt, kind="ExternalInput")
    wqk_d = nc.dram_tensor("wqk", [D, 2 * D], ddt, kind="ExternalInput")
    wv_d = nc.dram_tensor("wv", [D, D], ddt, kind="ExternalInput")
    wp_d = nc.dram_tensor("wp", [D, D], ddt, kind="ExternalInput")
    bias_d = nc.dram_tensor("bias", [P, D], dt.float32, kind="ExternalInput")
    ones_d = nc.dram_tensor("ones", [P, H, 1], pdt, kind="ExternalInput")
    out_d = nc.dram_tensor("out", [2, N, D], dt.float32,
                           kind="ExternalOutput")

    AUG = HD + 1  # 65: head dim + ones column for row sums

    with tile.TileContext(nc) as tc, ExitStack() as top:
        pp_s = top.enter_context(tc.tile_pool(name="ps_s", bufs=1,
                                              space="PSUM"))
        pp_o = top.enter_context(tc.tile_pool(name="ps_o", bufs=2,
                                              space="PSUM"))
        dram = top.enter_context(tc.tile_pool(name="dram", bufs=1,
                                              space="DRAM"))
        dram_rb = top.enter_context(tc.tile_pool(name="dram_rb", bufs=2,
                                                 space="DRAM"))
        persist = top.enter_context(tc.tile_pool(name="persist", bufs=1))
        pool_kv = top.enter_context(tc.tile_pool(name="kv", bufs=1))

        qT = persist.tile([P, NDT, N], ddt, tag="qT")
        wp_t = persist.tile([P, NDT, D], ddt, tag="wp")
        nc.sync.dma_start(
            out=wp_t, in_=wp_d[:].rearrange("(g p) d -> p g d", p=P))
        bias_t = persist.tile([P, D], dt.float32, tag="bias")
        nc.sync.dma_start(out=bias_t, in_=bias_d[:])

        k2_spill = dram.tile([NDT, P, N], ddt, tag="k2s")
        v2_spill = dram.tile([NQT, P, H, HD], pdt, tag="v2s")

        # ================= phase A: QKV (x then x2-with-spill) ==========
        kT = pool_kv.tile([P, NDT, N], ddt, tag="kT")
        vaug = pool_kv.tile([P, NQT, H * AUG], pdt, tag="vaug")

        def load_ones(vaug_t):
            for t in range(NQT):
                nc.sync.dma_start(
                    out=vaug_t[:, t, :].rearrange("p (h e) -> p h e",
                                                  e=AUG)[:, :, HD:AUG],
                    in_=ones_d[:])

        with tc.tile_pool(name="xa", bufs=2) as pool_x, \
             tc.tile_pool(name="wst", bufs=10) as pool_wst, \
             tc.tile_pool(name="wvp", bufs=1) as pool_wv, \
             tc.tile_pool(name="ev", bufs=3) as pool_ev:

            def qkv_T_form(xt_t, wcol0, dst_sb, dst_dram):
                """out-tiles [128, N] accumulated over in-tiles; PSUM evict
                on ScalarE (ACT idle in this phase)."""
                for o in range(NDT):
                    psf = pp_s.tile([P, 2, N] if STRUCT >= 1 else [P, N],
                                    dt.float32, tag="S")
                    ps = psf.rearrange("p (a n) -> p a n", a=1) \
                        if STRUCT < 1 else psf
                    for i in range(NDT):
                        wt = pool_wst.tile([P, P], ddt, tag="wt")
                        nc.sync.dma_start(
                            out=wt,
                            in_=wqk_d[i * P:(i + 1) * P,
                                      wcol0 + o * P: wcol0 + (o + 1) * P])
                        for c in range(2):
                            nc.tensor.matmul(
                                ps[:, 0, c * 512:(c + 1) * 512],
                                wt[:],
                                xt_t[:, i, c * 512:(c + 1) * 512],
                                start=(i == 0), stop=(i == NDT - 1))
                    cp = nc.scalar.copy if ACT_EVICT else nc.vector.tensor_copy
                    if dst_sb is not None:
                        cp(dst_sb[:, o, :], ps[:, 0, :])
                    else:
                        ev = pool_ev.tile([P, N], ddt, tag="ev")
                        cp(ev[:], ps[:, 0, :])
                        nc.sync.dma_start(out=dst_dram[o], in_=ev[:])

            def v_form(xt_t, wv_t, vaug_t, dst_dram):
                for t in range(NQT):
                    psf = pp_s.tile([P, 2, N] if STRUCT >= 1 else [P, N],
                                    dt.float32, tag="S")
                    ps = psf.rearrange("p (a n) -> p a n", a=1) \
                        if STRUCT < 1 else psf
                    for i in range(NDT):
                        for c0, cn in ((0, 512), (512, 256)):
                            nc.tensor.matmul(
                                ps[:, 0, c0:c0 + cn],
                                xt_t[:, i, t * P:(t + 1) * P],
                                wv_t[:, i, c0:c0 + cn],
                                start=(i == 0), stop=(i == NDT - 1))
                    src = ps[:, 0, 0:D].rearrange("p (h e) -> p h e", e=HD)
                    cp = nc.scalar.copy if ACT_EVICT else nc.vector.tensor_copy
                    if vaug_t is not None:
                        dstv = vaug_t[:, t, :].rearrange(
                            "p (h e) -> p h e", e=AUG)[:, :, 0:HD]
                        cp(dstv, src)
                    else:
                        ev = pool_ev.tile([P, H, HD], pdt, tag="evv")
                        cp(ev[:], src)
                        nc.sync.dma_start(out=dst_dram[t], in_=ev[:])

            xt_t = pool_x.tile([P, NDT, N], ddt, tag="xt")
            nc.sync.dma_start(out=xt_t,
                              in_=xt_d[:].rearrange("(i p) n -> p i n", p=P))
            wv_t = pool_wv.tile([P, NDT, D], ddt, tag="wv")
            nc.sync.dma_start(out=wv_t,
                              in_=wv_d[:].rearrange("(i p) d -> p i d", p=P))

            qkv_T_form(xt_t, 0, qT, None)            # qT
            qkv_T_form(xt_t, D, kT, None)            # kT
            load_ones(vaug)
            v_form(xt_t, wv_t, vaug, None)           # v -> vaug

            x2t_t = pool_x.tile([P, NDT, N], ddt, tag="xt")
            nc.sync.dma_start(out=x2t_t,
                              in_=x2t_d[:].rearrange("(i p) n -> p i n", p=P))
            qkv_T_form(x2t_t, D, None, k2_spill)     # k2T -> dram
            v_form(x2t_t, wv_t, None, v2_spill)      # v2 -> dram

        # ================= phase B: attention + proj ====================
        pool_pt = top.enter_context(tc.tile_pool(name="pt", bufs=1))
        pool_ot = top.enter_context(tc.tile_pool(name="ot", bufs=2))
        pool_res = top.enter_context(tc.tile_pool(name="res", bufs=2))
        pool_sm = top.enter_context(tc.tile_pool(name="sm", bufs=1))
        pool_osb = top.enter_context(tc.tile_pool(name="osb", bufs=2))

        def proj_qi(ot_t, br, qi):
            ps = pp_o.tile([P, D], dt.float32, tag="O")
            for g in range(NDT):
                for c0, cn in ((0, 512), (512, 256)):
                    nc.tensor.matmul(
                        ps[:, c0:c0 + cn],
                        ot_t[:, g, qi * P:(qi + 1) * P],
                        wp_t[:, g, c0:c0 + cn],
                        start=(g == 0), stop=(g == NDT - 1))
            res = pool_res.tile([P, D], dt.float32, tag="res")
            nc.vector.tensor_add(res[:], ps[:], bias_t[:])
            nc.sync.dma_start(out=out_d[br, qi * P:(qi + 1) * P, :],
                              in_=res[:])

        def attention(kT_t, vaug_t, br, extra):
            """head-pair ladder; STRUCT selects aggressiveness."""
            ot = pool_ot.tile([P, NDT, N], ddt, tag="ot")
            for g in range(NDT):
                pt2 = pool_pt.tile([P, 2, NQT, N], pdt, tag="pt")
                po = [pp_o.tile([AUG, N], dt.float32, tag="O",
                                name=f"po{br}_{g}_{hh}")
                      for hh in range(2)]

                def emit_av(kj):
                    for hh in range(2):
                        h = 2 * g + hh
                        for c in range(2):
                            nc.tensor.matmul(
                                po[hh][:, c * 512:(c + 1) * 512],
                                vaug_t[:, kj, h * AUG:(h + 1) * AUG],
                                pt2[:, hh, kj, c * 512:(c + 1) * 512],
                                start=(kj == 0), stop=(kj == NQT - 1),
                                skip_group_check=(STRUCT >= 1))

                for kj in range(NQT):
                    if STRUCT >= 1:
                        ps = pp_s.tile([P, 2, N], dt.float32, tag="S")
                        pse, pso = ps[:, 0, :], ps[:, 1, :]
                    else:
                        pse = pp_s.tile([P, N], dt.float32, tag="S")
                        pso = pp_s.tile([P, N], dt.float32, tag="S2")
                    for c in range(2):
                        nc.tensor.matmul(
                            pse[:, c * 512:(c + 1) * 512],
                            kT_t[0:HD, g, kj * P:(kj + 1) * P],
                            qT[0:HD, g, c * 512:(c + 1) * 512],
                            start=True, stop=True)
                        nc.tensor.matmul(
                            pso[:, c * 512:(c + 1) * 512],
                            kT_t[HD:P, g, kj * P:(kj + 1) * P],
                            qT[HD:P, g, c * 512:(c + 1) * 512],
                            start=True, stop=True)
                    if EXP_COMBINED and STRUCT >= 1:
                        nc.scalar.activation(
                            pt2[:, :, kj, :], ps[:],
                            mybir.ActivationFunctionType.Exp, scale=SCALE)
                    else:
                        nc.scalar.activation(
                            pt2[:, 0, kj, :], pse,
                            mybir.ActivationFunctionType.Exp, scale=SCALE)
                        nc.scalar.activation(
                            pt2[:, 1, kj, :], pso,
                            mybir.ActivationFunctionType.Exp, scale=SCALE)
                    if STRUCT >= 1 and kj % 4 == 3:
                        for kjb in range(kj - 3, kj + 1):
                            emit_av(kjb)
                if STRUCT == 0:
                    for kj in range(NQT):
                        emit_av(kj)
                for hh in range(2):
                    row = hh * HD
                    osb = pool_osb.tile([AUG, N], dt.float32, tag="osb")
                    nc.vector.tensor_copy(osb[:], po[hh][:])
                    r_t = pool_sm.tile([1, N], dt.float32, tag="r")
                    nc.vector.reciprocal(r_t[:], osb[HD:HD + 1, :])
                    r_bounce = dram_rb.tile([1, N], dt.float32, tag="rbb")
                    nc.sync.dma_start(out=r_bounce[:], in_=r_t[:])
                    rb_t = pool_sm.tile([HD, N], dt.float32, tag="rb")
                    nc.sync.dma_start(
                        out=rb_t[:],
                        in_=r_bounce[0, :].partition_broadcast(HD))
                    nc.vector.tensor_mul(
                        ot[row:row + HD, g, :], osb[0:HD, :], rb_t[:])
                if STRUCT >= 2 and extra:
                    extra.pop(0)()
                    if len(extra) > NDT - g - 1:
                        extra.pop(0)()
            while extra:
                extra.pop(0)()
            return ot

        ot0 = attention(kT, vaug, 0, [])

        # branch 2 k/v reload, then attention with proj(br0) interleaved
        kT2 = pool_kv.tile([P, NDT, N], ddt, tag="kT")
        nc.sync.dma_start(out=kT2,
                          in_=k2_spill[:].rearrange("g p n -> p g n"))
        vaug2 = pool_kv.tile([P, NQT, H * AUG], pdt, tag="vaug")
        load_ones(vaug2)
        for t in range(NQT):
            nc.sync.dma_start(
                out=vaug2[:, t, :].rearrange("p (h e) -> p h e",
                                             e=AUG)[:, :, 0:HD],
                in_=v2_spill[t])

        extra = ([(lambda qi=qi: proj_qi(ot0, 0, qi)) for qi in range(NQT)]
                 if STRUCT >= 2 else [])
        ot1 = attention(kT2, vaug2, 1, extra)
        if STRUCT < 2:
            for qi in range(NQT):
                proj_qi(ot0, 0, qi)
        for qi in range(NQT):
            proj_qi(ot1, 1, qi)

    n = _split_multiwaits(nc)
    _built = (nc, n)
    return _built


def _host_prep(x, x2, qkv_w, proj_w, proj_b):
    """-> list of 8 per-core input maps. Matmul operands are float32
    (device treats them as float32r); P/V-side constants are bfloat16."""
    import ml_dtypes
    f32 = lambda a: np.ascontiguousarray(a, dtype=np.float32)

    xt = np.ascontiguousarray(np.transpose(np.asarray(x), (0, 2, 1)))
    x2t = np.ascontiguousarray(np.transpose(np.asarray(x2), (0, 2, 1)))
    wqk = f32(np.asarray(qkv_w)[:2 * D].T)      # [768, 1536]
    wv = f32(np.asarray(qkv_w)[2 * D:].T)       # [768, 768]
    wp = f32(np.asarray(proj_w).T)              # [768, 768]
    bias = np.broadcast_to(np.asarray(proj_b, dtype=np.float32),
                           (P, D)).copy()
    ones = np.ones((P, H, 1), dtype=ml_dtypes.bfloat16)
    maps = []
    for c in range(B):
        maps.append({
            "xt": f32(xt[c]), "x2t": f32(x2t[c]),
            "wqk": wqk, "wv": wv, "wp": wp, "bias": bias,
            "ones": ones,
        })
    return maps


def kernel(x, x2, qkv_w, proj_w, proj_b, trace=False, tmpdir=None):
    nc, _ = _build()
    from concourse.bass_utils import run_bass_kernel_spmd
    in_maps = _host_prep(x, x2, qkv_w, proj_w, proj_b)
    res = run_bass_kernel_spmd(nc, in_maps, list(range(B)), trace=trace,
                               tmpdir=tmpdir)
    kernel.last_exec_time_ns = res.exec_time_ns
    out = np.stack([res.results[c]["out"] for c in range(B)])  # [B,2,N,D]
    out1 = np.ascontiguousarray(out[:, 0])
    out2 = np.ascontiguousarray(out[:, 1])
    return (out1, out2)


kernel.last_exec_time_ns = None



# revision 10
# speedup vs baseline: 1.5910x; 1.5910x over previous
"""Two-branch attention kernel for Trainium2 (8 NeuronCores, batch-parallel).

out1 = proj(softmax(q k^T / 8) v),  out2 = proj(softmax(q k2^T / 8) v2)
with q,k,v from x and k2,v2 from x2 (q shared across branches).

Sharding: batch dim (8) -> one batch element per core. No collectives.

Design (per core, all matmul operands bf16, PSUM f32):
  QKV:  qT/kT [dim,tok] via W-stationary matmuls; v [tok,dim] via
        x-stationary matmuls into a ones-augmented buffer (AUG=65 column
        carries softmax row sums for free). k2/v2 kept in SBUF (no DRAM
        spill); their formation is interleaved into branch-1 attention so
        the PE fills the ACT-bound softmax gaps.
  Attn: per (kj,c): S^T chunk [128,2,512] in a double-buffered PSUM pair;
        the two head-half matmuls land on PE row-groups 0/64 and run
        concurrently. exp on ScalarE (scale=1/8, no max subtraction)
        writes P^T bf16; AV (V_aug stationary) pipelined one kj behind.
  Norm: po -> osb evict on DVE; row sums bounce DRAM -> [128,2,8] so the
        reciprocal uses all partitions (~0.1us, not 6.5us); broadcast back
        and multiply into ot (bf16).
  Proj: ot tiles stationary, stream W_proj^T, add bias on DVE, DMA out.
        proj(br0) interleaved into branch-2 attention.
"""
import sys
for _p in ('/opt/trn_rl_repo',):
    if _p not in sys.path:
        sys.path.insert(0, _p)

import numpy as np

MODE = 'bf16'

B, N, D, H, HD = 8, 1024, 768, 12, 64
SCALE = HD ** -0.5
NDT = D // 128       # 6 dim tiles
NQT = N // 128       # 8 token tiles
P = 128
AUG = HD + 1         # 65: head dim + ones column for row sums


# ----------------------------------------------------------------------------
# workaround: walrus rejects >2 sem waits on one instruction; TileContext's
# tail drain carries one wait per active logical proc. Split them across
# single-wait SP nops and emit a bare drain.
def _install_tilefix():
    import bass_rust
    import concourse.tile as tile

    def _drain_and_barrier_split(self, tick_clock, wait_clock):
        gc = tick_clock.global_clock
        ticks = [gc[i] for i in range(27)]
        for i, t in enumerate(ticks):
            if t > 0:
                vc = bass_rust.VectorClock(
                    [t if j == i else 0 for j in range(len(ticks))])
                nop = self.nc.sync.nop()
                wait_clock.add_sem_waits(
                    nop.ins, bass_rust.ScopedClock({None: vc}))
        self.nc.sync.drain()
        self.nc.all_engine_barrier()
        assert self.sems is not None
        popped = self.nc._tile_sem_poison_stack.pop()
        assert popped is self._sem_poison
        self.nc.clear_and_free_semaphores(list(self.sems.allocated().values()))
        self.nc.all_engine_barrier()

    tile.TileContext._drain_and_barrier = _drain_and_barrier_split


def _split_multiwaits(nc, max_waits=1):
    """walrus codegen rejects instructions carrying more than `max_waits`
    sync waits; hoist the extras onto same-engine nops placed just before."""
    import bass_rust
    import concourse.mybir as mybir
    cnt = 0
    for bb in nc.main_func.blocks:
        insts = bb.instructions
        i = 0
        while i < len(insts):
            ins = insts[i]
            si = getattr(ins, 'sync_info', None)
            if si is not None and si.on_wait and len(si.on_wait) > max_waits:
                waits = list(si.on_wait)
                extras, keep = waits[:-max_waits], waits[-max_waits:]
                for w in extras:
                    nop = mybir.InstNoOp(name=f"I-swx{cnt}", ins=[], outs=[])
                    cnt += 1
                    nop.engine = ins.engine
                    nop.sync_info = bass_rust.SyncInfo(on_wait=[w],
                                                       on_update=[])
                    insts.insert(i, nop)
                    i += 1
                ins.sync_info = bass_rust.SyncInfo(
                    on_wait=keep, on_update=list(si.on_update))
            i += 1
    return cnt


_built = None


def _build():
    """Build the SPMD bass program once. Returns (nc, n_split_waits)."""
    global _built
    if _built is not None:
        return _built
    _install_tilefix()
    from contextlib import ExitStack
    import concourse.bass as bass
    import concourse.tile as tile
    from concourse import mybir

    dt = mybir.dt
    bf = dt.bfloat16

    nc = bass.Bass("TRN2", target_bir_lowering=False, debug=False,
                   num_devices=8)

    # DRAM I/O (per core)
    xt_d = nc.dram_tensor("xt", [D, N], bf, kind="ExternalInput")
    x2t_d = nc.dram_tensor("x2t", [D, N], bf, kind="ExternalInput")
    wqk_d = nc.dram_tensor("wqk", [D, 2 * D], bf, kind="ExternalInput")
    wv_d = nc.dram_tensor("wv", [D, D], bf, kind="ExternalInput")
    wp_d = nc.dram_tensor("wp", [D, D], bf, kind="ExternalInput")
    bias_d = nc.dram_tensor("bias", [P, D], dt.float32, kind="ExternalInput")
    ones_d = nc.dram_tensor("ones", [P, NQT * H], bf, kind="ExternalInput")
    out_d = nc.dram_tensor("out", [2, N, D], dt.float32,
                           kind="ExternalOutput")

    with tile.TileContext(nc) as tc, ExitStack() as top:
        pp_s = top.enter_context(tc.tile_pool(name="ps_s", bufs=2,
                                              space="PSUM"))
        pp_o = top.enter_context(tc.tile_pool(name="ps_o", bufs=2,
                                              space="PSUM"))
        dram_rb = top.enter_context(tc.tile_pool(name="dram_rb", bufs=4,
                                                 space="DRAM"))
        persist = top.enter_context(tc.tile_pool(name="persist", bufs=1))

        # persistent SBUF tiles
        qT = persist.tile([P, NDT, N], bf, tag="qT")
        kT1 = persist.tile([P, NDT, N], bf, tag="kT1")
        kT2 = persist.tile([P, NDT, N], bf, tag="kT2")
        vaug1 = persist.tile([P, NQT, H * AUG], bf, tag="vaug1")
        vaug2 = persist.tile([P, NQT, H * AUG], bf, tag="vaug2")
        KJR = 4          # pt2 ring depth over kj (AV trails exp by <=2)
        pt2 = persist.tile([P, 2, KJR, N], bf, tag="pt2")
        ot0 = persist.tile([P, NDT, N], bf, tag="ot0")
        ot1 = persist.tile([P, NDT, N], bf, tag="ot1")
        wp_t = persist.tile([P, NDT, D], bf, tag="wp")
        bias_t = persist.tile([P, D], dt.float32, tag="bias")

        nc.sync.dma_start(
            out=wp_t, in_=wp_d[:].rearrange("(g p) d -> p g d", p=P))
        nc.sync.dma_start(out=bias_t, in_=bias_d[:])

        def load_ones(vaug_t):
            nc.sync.dma_start(
                out=vaug_t[:].rearrange("p t (h e) -> p t h e",
                                        e=AUG)[:, :, :, HD:AUG],
                in_=ones_d[:].rearrange("p (t h e) -> p t h e", h=H, e=1))

        load_ones(vaug1)
        load_ones(vaug2)

        # ================= phase A pools (kept open through br0) =========
        pool_x = top.enter_context(tc.tile_pool(name="xa", bufs=2))
        pool_w = top.enter_context(tc.tile_pool(name="wqk", bufs=1))
        pool_wv = top.enter_context(tc.tile_pool(name="wvp", bufs=1))

        xt_t = pool_x.tile([P, NDT, N], bf, tag="xt")
        nc.sync.dma_start(out=xt_t,
                          in_=xt_d[:].rearrange("(i p) n -> p i n", p=P))
        wqk_t = pool_w.tile([P, NDT, 2 * D], bf, tag="wqk")
        nc.sync.dma_start(out=wqk_t,
                          in_=wqk_d[:].rearrange("(i p) d -> p i d", p=P))
        wv_t = pool_wv.tile([P, NDT, D], bf, tag="wv")
        nc.sync.dma_start(out=wv_t,
                          in_=wv_d[:].rearrange("(i p) d -> p i d", p=P))
        x2t_t = pool_x.tile([P, NDT, N], bf, tag="xt")
        nc.sync.dma_start(out=x2t_t,
                          in_=x2t_d[:].rearrange("(i p) n -> p i n", p=P))

        def qkv_T_o(xt_src, colblk, o, dst_sb, evict):
            """one output tile [128, N] of q/k-transposed formation."""
            ps = pp_s.tile([P, N], dt.float32, tag="S")
            for i in range(NDT):
                wt = wqk_t[:, i, colblk * D + o * P: colblk * D + (o + 1) * P]
                for c in range(2):
                    nc.tensor.matmul(
                        ps[:, c * 512:(c + 1) * 512],
                        wt,
                        xt_src[:, i, c * 512:(c + 1) * 512],
                        start=(i == 0), stop=(i == NDT - 1))
            evict(dst_sb[:, o, :], ps[:])

        def v_t(xt_src, vaug_t, t, evict):
            """one token tile [128, D] of v formation into vaug."""
            ps = pp_s.tile([P, D], dt.float32, tag="S")
            for i in range(NDT):
                for c0, cn in ((0, 512), (512, 256)):
                    nc.tensor.matmul(
                        ps[:, c0:c0 + cn],
                        xt_src[:, i, t * P:(t + 1) * P],
                        wv_t[:, i, c0:c0 + cn],
                        start=(i == 0), stop=(i == NDT - 1))
            dstv = vaug_t[:, t, :].rearrange(
                "p (h e) -> p h e", e=AUG)[:, :, 0:HD]
            evict(dstv, ps[:].rearrange("p (h e) -> p h e", e=HD))

        # phase A proper: q, k1, v1 (ACT evictions; ACT is free here)
        for o in range(NDT):
            qkv_T_o(xt_t, 0, o, qT, nc.scalar.copy)
        for o in range(NDT):
            qkv_T_o(xt_t, 1, o, kT1, nc.scalar.copy)
        for t in range(NQT):
            v_t(xt_t, vaug1, t, nc.scalar.copy)

        # k2/v2 thunks, interleaved into branch-1 attention (DVE evictions;
        # ACT is saturated by exp there)
        k2v2 = ([(lambda o=o: qkv_T_o(x2t_t, 1, o, kT2,
                                      nc.vector.tensor_copy))
                 for o in range(NDT)]
                + [(lambda t=t: v_t(x2t_t, vaug2, t, nc.vector.tensor_copy))
                   for t in range(NQT)])

        # ================= phase B: attention + proj ====================
        pool_osb = top.enter_context(tc.tile_pool(name="osb", bufs=4))
        pool_rc = top.enter_context(tc.tile_pool(name="rc", bufs=2))
        pool_rb = top.enter_context(tc.tile_pool(name="rb", bufs=2))
        pool_res = top.enter_context(tc.tile_pool(name="res", bufs=2))
        pool_otm = top.enter_context(tc.tile_pool(name="otm", bufs=2))

        def proj_qi(ot_t, br, qi):
            ps = pp_o.tile([P, D], dt.float32, tag="O")
            for g in range(NDT):
                for c0, cn in ((0, 512), (512, 256)):
                    nc.tensor.matmul(
                        ps[:, c0:c0 + cn],
                        ot_t[:, g, qi * P:(qi + 1) * P],
                        wp_t[:, g, c0:c0 + cn],
                        start=(g == 0), stop=(g == NDT - 1))
            res = pool_res.tile([P, D], dt.float32, tag="res")
            nc.vector.tensor_add(res[:], ps[:], bias_t[:])
            nc.sync.dma_start(out=out_d[br, qi * P:(qi + 1) * P, :],
                              in_=res[:])

        def attention(kT_t, vaug_t, ot, extra, budget):
            """head-pair ladder over g; `extra` thunks fill PE slack.
            `budget` = max extra thunks to pop per g iteration."""
            for g in range(NDT):
                po = [pp_o.tile([AUG, N], dt.float32, tag="O",
                                name=f"po{g}_{hh}") for hh in range(2)]

                def emit_av(kj):
                    for hh in range(2):
                        h = 2 * g + hh
                        for c in range(2):
                            nc.tensor.matmul(
                                po[hh][:, c * 512:(c + 1) * 512],
                                vaug_t[:, kj, h * AUG:(h + 1) * AUG],
                                pt2[:, hh, kj % KJR, c * 512:(c + 1) * 512],
                                start=(kj == 0), stop=(kj == NQT - 1),
                                skip_group_check=True)

                for kj in range(NQT):
                    for c in range(2):
                        ps = pp_s.tile([P, 2, 512], dt.float32, tag="S")
                        nc.tensor.matmul(
                            ps[:, 0, :],
                            kT_t[0:HD, g, kj * P:(kj + 1) * P],
                            qT[0:HD, g, c * 512:(c + 1) * 512],
                            start=True, stop=True)
                        nc.tensor.matmul(
                            ps[:, 1, :],
                            kT_t[HD:P, g, kj * P:(kj + 1) * P],
                            qT[HD:P, g, c * 512:(c + 1) * 512],
                            start=True, stop=True)
                        nc.scalar.activation(
                            pt2[:, :, kj % KJR, c * 512:(c + 1) * 512],
                            ps[:],
                            mybir.ActivationFunctionType.Exp, scale=SCALE)
                    if kj >= 1:
                        emit_av(kj - 1)
                emit_av(NQT - 1)

                # evict po fast (frees PSUM), then normalize off-band
                osb = [pool_osb.tile([AUG, N], dt.float32, tag="osb",
                                     name=f"osb{g}_{hh}")
                       for hh in range(2)]
                for hh in range(2):
                    nc.vector.tensor_copy(osb[hh][:], po[hh][:])
                rdrm = dram_rb.tile([2, N], dt.float32, tag="rd")
                for hh in range(2):
                    nc.sync.dma_start(out=rdrm[hh, :],
                                      in_=osb[hh][HD:HD + 1, :])
                rcol = pool_rc.tile([P, 2, 8], dt.float32, tag="rc")
                nc.sync.dma_start(
                    out=rcol,
                    in_=rdrm[:].rearrange("a (p c) -> p a c", p=P))
                rcol2 = pool_rc.tile([P, 2, 8], dt.float32, tag="rc2")
                nc.vector.reciprocal(rcol2[:], rcol[:])
                rdrm2 = dram_rb.tile([2, N], dt.float32, tag="rd2")
                nc.sync.dma_start(
                    out=rdrm2[:].rearrange("a (p c) -> p a c", p=P),
                    in_=rcol2)
                rb = pool_rb.tile([HD, 2, N], dt.float32, tag="rb")
                nc.sync.dma_start(out=rb[:, 0, :],
                                  in_=rdrm2[0, :].partition_broadcast(HD))
                nc.sync.dma_start(out=rb[:, 1, :],
                                  in_=rdrm2[1, :].partition_broadcast(HD))
                # DVE operands must share partitions 0:HD; hh=1's result is
                # partition-shifted into ot[HD:] by DMA.
                nc.vector.tensor_mul(ot[0:HD, g, :], osb[0][0:HD, :],
                                     rb[:, 0, :])
                otm = pool_otm.tile([HD, N], bf, tag="otm")
                nc.vector.tensor_mul(otm[:], osb[1][0:HD, :], rb[:, 1, :])
                nc.sync.dma_start(out=ot[HD:P, g, :], in_=otm[:])

                for _ in range(budget):
                    if extra:
                        extra.pop(0)()
            while extra:
                extra.pop(0)()

        attention(kT1, vaug1, ot0, k2v2, budget=3)
        extra = [(lambda qi=qi: proj_qi(ot0, 0, qi)) for qi in range(NQT)]
        attention(kT2, vaug2, ot1, extra, budget=2)
        for qi in range(NQT):
            proj_qi(ot1, 1, qi)

    n = _split_multiwaits(nc)
    _built = (nc, n)
    return _built


def _host_prep(x, x2, qkv_w, proj_w, proj_b):
    """-> list of 8 per-core input maps (bf16 operands, f32 bias)."""
    import ml_dtypes
    b16 = lambda a: np.ascontiguousarray(a).astype(ml_dtypes.bfloat16)

    xt = np.transpose(np.asarray(x), (0, 2, 1))
    x2t = np.transpose(np.asarray(x2), (0, 2, 1))
    wqk = b16(np.asarray(qkv_w)[:2 * D].T)      # [768, 1536]
    wv = b16(np.asarray(qkv_w)[2 * D:].T)       # [768, 768]
    wp = b16(np.asarray(proj_w).T)              # [768, 768]
    bias = np.broadcast_to(np.asarray(proj_b, dtype=np.float32),
                           (P, D)).copy()
    ones = np.ones((P, NQT * H), dtype=ml_dtypes.bfloat16)
    maps = []
    for c in range(B):
        maps.append({
            "xt": b16(xt[c]), "x2t": b16(x2t[c]),
            "wqk": wqk, "wv": wv, "wp": wp, "bias": bias,
            "ones": ones,
        })
    return maps


def kernel(x, x2, qkv_w, proj_w, proj_b, trace=False, tmpdir=None):
    nc, _ = _build()
    from concourse.bass_utils import run_bass_kernel_spmd
    in_maps = _host_prep(x, x2, qkv_w, proj_w, proj_b)
    res = run_bass_kernel_spmd(nc, in_maps, list(range(B)), trace=trace,
                               tmpdir=tmpdir)
    kernel.last_exec_time_ns = res.exec_time_ns
    out = np.stack([res.results[c]["out"] for c in range(B)])  # [B,2,N,D]
    out1 = np.ascontiguousarray(out[:, 0])
    out2 = np.ascontiguousarray(out[:, 1])
    return (out1, out2)


kernel.last_exec_time_ns = None


# revision 19
# speedup vs baseline: 1.6492x; 1.0365x over previous
"""Two-branch attention kernel for Trainium2 (8 NeuronCores, batch-parallel).

out1 = proj(softmax(q k^T / 8) v),  out2 = proj(softmax(q k2^T / 8) v2)
with q,k,v from x and k2,v2 from x2 (q shared across branches).

Sharding: batch dim (8) -> one batch element per core. No collectives.

Design (per core, all matmul operands bf16, PSUM f32):
  QKV:  qT/kT [dim,tok] via W-stationary matmuls; v [tok,dim] via
        x-stationary matmuls into a ones-augmented buffer (AUG=65 column
        carries softmax row sums for free). k2/v2 kept in SBUF (no DRAM
        spill); their formation is interleaved into branch-1 attention so
        the PE fills the ACT-bound softmax gaps.
  Attn: per (kj,c): S^T chunk [128,2,512] in a double-buffered PSUM pair;
        the two head-half matmuls land on PE row-groups 0/64 and run
        concurrently. exp on ScalarE (scale=1/8, no max subtraction)
        writes P^T bf16; AV (V_aug stationary) pipelined one kj behind.
  Norm: po -> osb evict on DVE; row sums bounce DRAM -> [128,2,8] so the
        reciprocal uses all partitions (~0.1us, not 6.5us); broadcast back
        and multiply into ot (bf16).
  Proj: ot tiles stationary, stream W_proj^T, add bias on DVE, DMA out.
        proj(br0) interleaved into branch-2 attention.
"""
import sys
for _p in ('/opt/trn_rl_repo',):
    if _p not in sys.path:
        sys.path.insert(0, _p)

import numpy as np

MODE = 'bf16'

B, N, D, H, HD = 8, 1024, 768, 12, 64
SCALE = HD ** -0.5
NDT = D // 128       # 6 dim tiles
NQT = N // 128       # 8 token tiles
P = 128
AUG = HD + 1         # 65: head dim + ones column for row sums


# ----------------------------------------------------------------------------
# workaround: walrus rejects >2 sem waits on one instruction; TileContext's
# tail drain carries one wait per active logical proc. Split them across
# single-wait SP nops and emit a bare drain.
def _install_tilefix():
    import bass_rust
    import concourse.tile as tile

    def _drain_and_barrier_split(self, tick_clock, wait_clock):
        gc = tick_clock.global_clock
        ticks = [gc[i] for i in range(27)]
        for i, t in enumerate(ticks):
            if t > 0:
                vc = bass_rust.VectorClock(
                    [t if j == i else 0 for j in range(len(ticks))])
                nop = self.nc.sync.nop()
                wait_clock.add_sem_waits(
                    nop.ins, bass_rust.ScopedClock({None: vc}))
        self.nc.sync.drain()
        self.nc.all_engine_barrier()
        assert self.sems is not None
        popped = self.nc._tile_sem_poison_stack.pop()
        assert popped is self._sem_poison
        self.nc.clear_and_free_semaphores(list(self.sems.allocated().values()))
        self.nc.all_engine_barrier()

    tile.TileContext._drain_and_barrier = _drain_and_barrier_split


def _split_multiwaits(nc, max_waits=1):
    """walrus codegen rejects instructions carrying more than `max_waits`
    sync waits; hoist the extras onto same-engine nops placed just before."""
    import bass_rust
    import concourse.mybir as mybir
    cnt = 0
    for bb in nc.main_func.blocks:
        insts = bb.instructions
        i = 0
        while i < len(insts):
            ins = insts[i]
            si = getattr(ins, 'sync_info', None)
            if si is not None and si.on_wait and len(si.on_wait) > max_waits:
                waits = list(si.on_wait)
                extras, keep = waits[:-max_waits], waits[-max_waits:]
                for w in extras:
                    nop = mybir.InstNoOp(name=f"I-swx{cnt}", ins=[], outs=[])
                    cnt += 1
                    nop.engine = ins.engine
                    nop.sync_info = bass_rust.SyncInfo(on_wait=[w],
                                                       on_update=[])
                    insts.insert(i, nop)
                    i += 1
                ins.sync_info = bass_rust.SyncInfo(
                    on_wait=keep, on_update=list(si.on_update))
            i += 1
    return cnt


_built = None


def _build():
    """Build the SPMD bass program once. Returns (nc, n_split_waits)."""
    global _built
    if _built is not None:
        return _built
    _install_tilefix()
    from contextlib import ExitStack
    import concourse.bass as bass
    import concourse.tile as tile
    from concourse import mybir

    dt = mybir.dt
    bf = dt.bfloat16

    nc = bass.Bass("TRN2", target_bir_lowering=False, debug=False,
                   num_devices=8)

    # DRAM I/O (per core)
    xt_d = nc.dram_tensor("xt", [D, N], bf, kind="ExternalInput")
    x2t_d = nc.dram_tensor("x2t", [D, N], bf, kind="ExternalInput")
    wqk_d = nc.dram_tensor("wqk", [D, 2 * D], bf, kind="ExternalInput")
    wv_d = nc.dram_tensor("wv", [D, D], bf, kind="ExternalInput")
    wp_d = nc.dram_tensor("wp", [D, D], bf, kind="ExternalInput")
    bias_d = nc.dram_tensor("bias", [P, D], dt.float32, kind="ExternalInput")
    ones_d = nc.dram_tensor("ones", [P, NQT * H], bf, kind="ExternalInput")
    out_d = nc.dram_tensor("out", [2, N, D], dt.float32,
                           kind="ExternalOutput")

    with tile.TileContext(nc) as tc, ExitStack() as top:
        pp_s = top.enter_context(tc.tile_pool(name="ps_s", bufs=2,
                                              space="PSUM"))
        pp_o = top.enter_context(tc.tile_pool(name="ps_o", bufs=2,
                                              space="PSUM"))
        dram_rb = top.enter_context(tc.tile_pool(name="dram_rb", bufs=4,
                                                 space="DRAM"))
        persist = top.enter_context(tc.tile_pool(name="persist", bufs=1))

        # persistent SBUF tiles
        qT = persist.tile([P, NDT, N], bf, tag="qT")
        kT1 = persist.tile([P, NDT, N], bf, tag="kT1")
        kT2 = persist.tile([P, NDT, N], bf, tag="kT2")
        vaug1 = persist.tile([P, NQT, H * AUG], bf, tag="vaug1")
        vaug2 = persist.tile([P, NQT, H * AUG], bf, tag="vaug2")
        KJR = 4          # pt2 ring depth over kj (AV trails exp by <=2)
        pt2 = persist.tile([P, 2, KJR, N], bf, tag="pt2")
        ot0 = persist.tile([P, NDT, N], bf, tag="ot0")
        ot1 = persist.tile([P, NDT, N], bf, tag="ot1")
        wp_t = persist.tile([P, NDT, D], bf, tag="wp")
        bias_t = persist.tile([P, D], dt.float32, tag="bias")

        def load_ones(vaug_t):
            nc.sync.dma_start(
                out=vaug_t[:].rearrange("p t (h e) -> p t h e",
                                        e=AUG)[:, :, :, HD:AUG],
                in_=ones_d[:].rearrange("p (t h e) -> p t h e", h=H, e=1))

        # ================= phase A pools (kept open through br0) =========
        pool_x = top.enter_context(tc.tile_pool(name="xa", bufs=2))
        pool_w = top.enter_context(tc.tile_pool(name="wqk", bufs=1))
        pool_wv = top.enter_context(tc.tile_pool(name="wvp", bufs=1))

        # input DMAs split per tile so the first matmul only waits on xt +
        # wqk's first column block; the rest stream under compute.
        xt_t = pool_x.tile([P, NDT, N], bf, tag="xt")
        xt_r = xt_d[:].rearrange("(i p) n -> p i n", p=P)
        for i in range(NDT):
            nc.sync.dma_start(out=xt_t[:, i, :], in_=xt_r[:, i, :])
        wqk_t = pool_w.tile([P, NDT, 2 * D], bf, tag="wqk")
        wqk_r = wqk_d[:].rearrange("(i p) d -> p i d", p=P)
        # blocks 0 (q o=0) and 6 (k o=0) first: the serial prefix needs them
        for o in [0, NDT] + [o for o in range(2 * NDT) if o not in (0, NDT)]:
            nc.sync.dma_start(out=wqk_t[:, :, o * P:(o + 1) * P],
                              in_=wqk_r[:, :, o * P:(o + 1) * P])
        load_ones(vaug1)
        wv_t = pool_wv.tile([P, NDT, D], bf, tag="wv")
        wv_r = wv_d[:].rearrange("(i p) d -> p i d", p=P)
        for i in range(NDT):
            nc.sync.dma_start(out=wv_t[:, i, :], in_=wv_r[:, i, :])
        x2t_t = pool_x.tile([P, NDT, N], bf, tag="xt")
        x2t_r = x2t_d[:].rearrange("(i p) n -> p i n", p=P)
        for i in range(NDT):
            nc.sync.dma_start(out=x2t_t[:, i, :], in_=x2t_r[:, i, :])
        load_ones(vaug2)
        nc.sync.dma_start(
            out=wp_t, in_=wp_d[:].rearrange("(g p) d -> p g d", p=P))
        nc.sync.dma_start(out=bias_t, in_=bias_d[:])

        def qkv_T_o(xt_src, colblk, o, dst_sb, evict):
            """one output tile [128, N] of q/k-transposed formation."""
            ps = pp_s.tile([P, N], dt.float32, tag="S")
            for i in range(NDT):
                wt = wqk_t[:, i, colblk * D + o * P: colblk * D + (o + 1) * P]
                for c in range(2):
                    nc.tensor.matmul(
                        ps[:, c * 512:(c + 1) * 512],
                        wt,
                        xt_src[:, i, c * 512:(c + 1) * 512],
                        start=(i == 0), stop=(i == NDT - 1))
            evict(dst_sb[:, o, :], ps[:])

        def v_t(xt_src, vaug_t, t, evict):
            """one token tile [128, D] of v formation into vaug."""
            ps = pp_s.tile([P, D], dt.float32, tag="S")
            for i in range(NDT):
                for c0, cn in ((0, 512), (512, 256)):
                    nc.tensor.matmul(
                        ps[:, c0:c0 + cn],
                        xt_src[:, i, t * P:(t + 1) * P],
                        wv_t[:, i, c0:c0 + cn],
                        start=(i == 0), stop=(i == NDT - 1))
            dstv = vaug_t[:, t, :].rearrange(
                "p (h e) -> p h e", e=AUG)[:, :, 0:HD]
            evict(dstv, ps[:].rearrange("p (h e) -> p h e", e=HD))

        # phase A serial prefix: only what branch-0's first g needs —
        # q(o=0), k1(o=0), all of v1 (AV consumes every token tile in g=0).
        # ACT evictions (ACT is free here).
        qkv_T_o(xt_t, 0, 0, qT, nc.scalar.copy)
        qkv_T_o(xt_t, 1, 0, kT1, nc.scalar.copy)
        for t in range(NQT):
            v_t(xt_t, vaug1, t, nc.scalar.copy)

        # remaining QKV work rides branch-0 attention's PE slack (the g loop
        # is ACT-bound): q/k1 tile o is needed by g=o, so pairs go first in
        # order; k2/v2 must finish before branch-1 starts. DVE evictions
        # (ACT is saturated by exp there).
        thunks0 = []
        for o in range(1, NDT):
            thunks0.append(lambda o=o: qkv_T_o(xt_t, 0, o, qT,
                                               nc.vector.tensor_copy))
            thunks0.append(lambda o=o: qkv_T_o(xt_t, 1, o, kT1,
                                               nc.vector.tensor_copy))
        for o in range(NDT):
            thunks0.append(lambda o=o: qkv_T_o(x2t_t, 1, o, kT2,
                                               nc.vector.tensor_copy))
        for t in range(NQT):
            thunks0.append(lambda t=t: v_t(x2t_t, vaug2, t,
                                           nc.vector.tensor_copy))

        # ================= phase B: attention + proj ====================
        pool_osb = top.enter_context(tc.tile_pool(name="osb", bufs=4))
        pool_rc = top.enter_context(tc.tile_pool(name="rc", bufs=4))
        pool_rb = top.enter_context(tc.tile_pool(name="rb", bufs=2))
        pool_res = top.enter_context(tc.tile_pool(name="res", bufs=2))
        pool_otm = top.enter_context(tc.tile_pool(name="otm", bufs=2))

        def proj_qi(ot_t, br, qi):
            ps = pp_o.tile([P, D], dt.float32, tag="O")
            for g in range(NDT):
                for c0, cn in ((0, 512), (512, 256)):
                    nc.tensor.matmul(
                        ps[:, c0:c0 + cn],
                        ot_t[:, g, qi * P:(qi + 1) * P],
                        wp_t[:, g, c0:c0 + cn],
                        start=(g == 0), stop=(g == NDT - 1))
            res = pool_res.tile([P, D], dt.float32, tag="res")
            nc.vector.tensor_add(res[:], ps[:], bias_t[:])
            nc.sync.dma_start(out=out_d[br, qi * P:(qi + 1) * P, :],
                              in_=res[:])

        def attention(kT_t, vaug_t, ot, extra, budget):
            """head-pair ladder over g; `extra` thunks fill PE slack.
            `budget` = max extra thunks to pop per g iteration. The
            normalize tail of g is deferred one iteration so its DVE ops
            never sit in front of g+1's PSUM eviction in the FIFO (the rb
            broadcast's DMA latency is hidden by then)."""
            pending = []
            for g in range(NDT):
                po = [pp_o.tile([AUG, N], dt.float32, tag="O",
                                name=f"po{g}_{hh}") for hh in range(2)]

                def emit_av(kj):
                    for hh in range(2):
                        h = 2 * g + hh
                        for c in range(2):
                            nc.tensor.matmul(
                                po[hh][:, c * 512:(c + 1) * 512],
                                vaug_t[:, kj, h * AUG:(h + 1) * AUG],
                                pt2[:, hh, kj % KJR, c * 512:(c + 1) * 512],
                                start=(kj == 0), stop=(kj == NQT - 1),
                                skip_group_check=True)

                for kj in range(NQT):
                    for c in range(2):
                        ps = pp_s.tile([P, 2, 512], dt.float32, tag="S")
                        nc.tensor.matmul(
                            ps[:, 0, :],
                            kT_t[0:HD, g, kj * P:(kj + 1) * P],
                            qT[0:HD, g, c * 512:(c + 1) * 512],
                            start=True, stop=True)
                        nc.tensor.matmul(
                            ps[:, 1, :],
                            kT_t[HD:P, g, kj * P:(kj + 1) * P],
                            qT[HD:P, g, c * 512:(c + 1) * 512],
                            start=True, stop=True)
                        nc.scalar.activation(
                            pt2[:, :, kj % KJR, c * 512:(c + 1) * 512],
                            ps[:],
                            mybir.ActivationFunctionType.Exp, scale=SCALE)
                    if kj >= 1:
                        emit_av(kj - 1)
                emit_av(NQT - 1)

                # evict po fast (frees PSUM) and kick off the r reshape
                osb = [pool_osb.tile([AUG, N], dt.float32, tag="osb",
                                     name=f"osb{g}_{hh}")
                       for hh in range(2)]
                for hh in range(2):
                    nc.vector.tensor_copy(osb[hh][:], po[hh][:])
                rdrm = dram_rb.tile([2, N], dt.float32, tag="rd")
                for hh in range(2):
                    nc.sync.dma_start(out=rdrm[hh, :],
                                      in_=osb[hh][HD:HD + 1, :])
                rcol = pool_rc.tile([P, 2, 8], dt.float32, tag="rc")
                nc.sync.dma_start(
                    out=rcol,
                    in_=rdrm[:].rearrange("a (p c) -> p a c", p=P))

                def normalize(g=g, osb=osb, rcol=rcol):
                    rcol2 = pool_rc.tile([P, 2, 8], dt.float32, tag="rc2")
                    nc.vector.reciprocal(rcol2[:], rcol[:])
                    rdrm2 = dram_rb.tile([2, N], dt.float32, tag="rd2")
                    nc.sync.dma_start(
                        out=rdrm2[:].rearrange("a (p c) -> p a c", p=P),
                        in_=rcol2)
                    rb = pool_rb.tile([HD, 2, N], dt.float32, tag="rb")
                    nc.sync.dma_start(
                        out=rb[:, 0, :],
                        in_=rdrm2[0, :].partition_broadcast(HD))
                    nc.sync.dma_start(
                        out=rb[:, 1, :],
                        in_=rdrm2[1, :].partition_broadcast(HD))
                    # DVE operands must share partitions 0:HD; hh=1's
                    # result is partition-shifted into ot[HD:] by DMA.
                    nc.vector.tensor_mul(ot[0:HD, g, :], osb[0][0:HD, :],
                                         rb[:, 0, :])
                    otm = pool_otm.tile([HD, N], bf, tag="otm")
                    nc.vector.tensor_mul(otm[:], osb[1][0:HD, :],
                                         rb[:, 1, :])
                    nc.sync.dma_start(out=ot[HD:P, g, :], in_=otm[:])

                pending.append(normalize)
                if len(pending) > 1:
                    pending.pop(0)()
                for _ in range(budget):
                    if extra:
                        extra.pop(0)()
            while extra:
                extra.pop(0)()
            while pending:
                pending.pop(0)()

        attention(kT1, vaug1, ot0, thunks0, budget=5)
        # 6 proj(br0) tiles ride branch-1; 2 are held back to keep the PE
        # warm through branch-1's final normalize chain.
        extra = [(lambda qi=qi: proj_qi(ot0, 0, qi)) for qi in range(NQT - 2)]
        attention(kT2, vaug2, ot1, extra, budget=1)
        for qi in range(NQT - 2, NQT):
            proj_qi(ot0, 0, qi)
        for qi in range(NQT):
            proj_qi(ot1, 1, qi)

    n = _split_multiwaits(nc)
    _built = (nc, n)
    return _built


def _host_prep(x, x2, qkv_w, proj_w, proj_b):
    """-> list of 8 per-core input maps (bf16 operands, f32 bias)."""
    import ml_dtypes
    b16 = lambda a: np.ascontiguousarray(a).astype(ml_dtypes.bfloat16)

    xt = np.transpose(np.asarray(x), (0, 2, 1))
    x2t = np.transpose(np.asarray(x2), (0, 2, 1))
    wqk = b16(np.asarray(qkv_w)[:2 * D].T)      # [768, 1536]
    wv = b16(np.asarray(qkv_w)[2 * D:].T)       # [768, 768]
    wp = b16(np.asarray(proj_w).T)              # [768, 768]
    bias = np.broadcast_to(np.asarray(proj_b, dtype=np.float32),
                           (P, D)).copy()
    ones = np.ones((P, NQT * H), dtype=ml_dtypes.bfloat16)
    maps = []
    for c in range(B):
        maps.append({
            "xt": b16(xt[c]), "x2t": b16(x2t[c]),
            "wqk": wqk, "wv": wv, "wp": wp, "bias": bias,
            "ones": ones,
        })
    return maps


def kernel(x, x2, qkv_w, proj_w, proj_b, trace=False, tmpdir=None):
    nc, _ = _build()
    from concourse.bass_utils import run_bass_kernel_spmd
    in_maps = _host_prep(x, x2, qkv_w, proj_w, proj_b)
    res = run_bass_kernel_spmd(nc, in_maps, list(range(B)), trace=trace,
                               tmpdir=tmpdir)
    kernel.last_exec_time_ns = res.exec_time_ns
    out = np.stack([res.results[c]["out"] for c in range(B)])  # [B,2,N,D]
    out1 = np.ascontiguousarray(out[:, 0])
    out2 = np.ascontiguousarray(out[:, 1])
    return (out1, out2)


kernel.last_exec_time_ns = None


# revision 25
# speedup vs baseline: 1.6906x; 1.0251x over previous
"""Two-branch attention kernel for Trainium2 (8 NeuronCores, batch-parallel).

out1 = proj(softmax(q k^T / 8) v),  out2 = proj(softmax(q k2^T / 8) v2)
with q,k,v from x and k2,v2 from x2 (q shared across branches).

Sharding: batch dim (8) -> one batch element per core. No collectives.

Design (per core, all matmul operands bf16, PSUM f32):
  QKV:  qT/kT [dim,tok] via W-stationary matmuls; v [tok,dim] via
        x-stationary matmuls into a ones-augmented buffer (AUG=65 column
        carries softmax row sums for free). k2/v2 kept in SBUF (no DRAM
        spill); their formation is interleaved into branch-1 attention so
        the PE fills the ACT-bound softmax gaps.
  Attn: per (kj,c): S^T chunk [128,2,512] in a double-buffered PSUM pair;
        the two head-half matmuls land on PE row-groups 0/64 and run
        concurrently. exp on ScalarE (scale=1/8, no max subtraction)
        writes P^T bf16; AV (V_aug stationary) pipelined one kj behind.
  Norm: po -> osb evict on DVE; row sums bounce DRAM -> [128,2,8] so the
        reciprocal uses all partitions (~0.1us, not 6.5us); broadcast back
        and multiply into ot (bf16).
  Proj: ot tiles stationary, stream W_proj^T, add bias on DVE, DMA out.
        proj(br0) interleaved into branch-2 attention.
"""
import sys
for _p in ('/opt/trn_rl_repo',):
    if _p not in sys.path:
        sys.path.insert(0, _p)

import numpy as np

MODE = 'bf16'

B, N, D, H, HD = 8, 1024, 768, 12, 64
SCALE = HD ** -0.5
NDT = D // 128       # 6 dim tiles
NQT = N // 128       # 8 token tiles
P = 128
AUG = HD + 1         # 65: head dim + ones column for row sums


# ----------------------------------------------------------------------------
# workaround: walrus rejects >2 sem waits on one instruction; TileContext's
# tail drain carries one wait per active logical proc. Split them across
# single-wait SP nops and emit a bare drain.
def _install_tilefix():
    import bass_rust
    import concourse.tile as tile

    def _drain_and_barrier_split(self, tick_clock, wait_clock):
        gc = tick_clock.global_clock
        ticks = [gc[i] for i in range(27)]
        for i, t in enumerate(ticks):
            if t > 0:
                vc = bass_rust.VectorClock(
                    [t if j == i else 0 for j in range(len(ticks))])
                nop = self.nc.sync.nop()
                wait_clock.add_sem_waits(
                    nop.ins, bass_rust.ScopedClock({None: vc}))
        self.nc.sync.drain()
        self.nc.all_engine_barrier()
        assert self.sems is not None
        popped = self.nc._tile_sem_poison_stack.pop()
        assert popped is self._sem_poison
        self.nc.clear_and_free_semaphores(list(self.sems.allocated().values()))
        self.nc.all_engine_barrier()

    tile.TileContext._drain_and_barrier = _drain_and_barrier_split


def _split_multiwaits(nc, max_waits=1):
    """walrus codegen rejects instructions carrying more than `max_waits`
    sync waits; hoist the extras onto same-engine nops placed just before."""
    import bass_rust
    import concourse.mybir as mybir
    cnt = 0
    for bb in nc.main_func.blocks:
        insts = bb.instructions
        i = 0
        while i < len(insts):
            ins = insts[i]
            si = getattr(ins, 'sync_info', None)
            if si is not None and si.on_wait and len(si.on_wait) > max_waits:
                waits = list(si.on_wait)
                extras, keep = waits[:-max_waits], waits[-max_waits:]
                for w in extras:
                    nop = mybir.InstNoOp(name=f"I-swx{cnt}", ins=[], outs=[])
                    cnt += 1
                    nop.engine = ins.engine
                    nop.sync_info = bass_rust.SyncInfo(on_wait=[w],
                                                       on_update=[])
                    insts.insert(i, nop)
                    i += 1
                ins.sync_info = bass_rust.SyncInfo(
                    on_wait=keep, on_update=list(si.on_update))
            i += 1
    return cnt


_built = None


def _build():
    """Build the SPMD bass program once. Returns (nc, n_split_waits)."""
    global _built
    if _built is not None:
        return _built
    _install_tilefix()
    from contextlib import ExitStack
    import concourse.bass as bass
    import concourse.tile as tile
    from concourse import mybir

    dt = mybir.dt
    bf = dt.bfloat16

    nc = bass.Bass("TRN2", target_bir_lowering=False, debug=False,
                   num_devices=8)

    # DRAM I/O (per core)
    xt_d = nc.dram_tensor("xt", [D, N], bf, kind="ExternalInput")
    x2t_d = nc.dram_tensor("x2t", [D, N], bf, kind="ExternalInput")
    wqk_d = nc.dram_tensor("wqk", [D, 2 * D], bf, kind="ExternalInput")
    wv_d = nc.dram_tensor("wv", [D, D], bf, kind="ExternalInput")
    wp_d = nc.dram_tensor("wp", [D, D], bf, kind="ExternalInput")
    bias_d = nc.dram_tensor("bias", [P, D], dt.float32, kind="ExternalInput")
    ones_d = nc.dram_tensor("ones", [P, NQT * H], bf, kind="ExternalInput")
    out_d = nc.dram_tensor("out", [2, N, D], dt.float32,
                           kind="ExternalOutput")

    with tile.TileContext(nc) as tc, ExitStack() as top:
        pp_s = top.enter_context(tc.tile_pool(name="ps_s", bufs=2,
                                              space="PSUM"))
        pp_o = top.enter_context(tc.tile_pool(name="ps_o", bufs=2,
                                              space="PSUM"))
        dram_rb = top.enter_context(tc.tile_pool(name="dram_rb", bufs=4,
                                                 space="DRAM"))
        persist = top.enter_context(tc.tile_pool(name="persist", bufs=1))

        # persistent SBUF tiles
        qT = persist.tile([P, NDT, N], bf, tag="qT")
        kT1 = persist.tile([P, NDT, N], bf, tag="kT1")
        kT2 = persist.tile([P, NDT, N], bf, tag="kT2")
        vaug1 = persist.tile([P, NQT, H * AUG], bf, tag="vaug1")
        vaug2 = persist.tile([P, NQT, H * AUG], bf, tag="vaug2")
        KJR = 4          # pt2 ring depth over kj (AV trails exp by <=2)
        pt2 = persist.tile([P, 2, KJR, N], bf, tag="pt2")
        ot0 = persist.tile([P, NDT, N], bf, tag="ot0")
        ot1 = persist.tile([P, NDT, N], bf, tag="ot1")
        wp_t = persist.tile([P, NDT, D], bf, tag="wp")
        bias_t = persist.tile([P, D], dt.float32, tag="bias")

        def load_ones(vaug_t):
            nc.sync.dma_start(
                out=vaug_t[:].rearrange("p t (h e) -> p t h e",
                                        e=AUG)[:, :, :, HD:AUG],
                in_=ones_d[:].rearrange("p (t h e) -> p t h e", h=H, e=1))

        # ================= phase A pools (kept open through br0) =========
        pool_x = top.enter_context(tc.tile_pool(name="xa", bufs=2))
        pool_w = top.enter_context(tc.tile_pool(name="wqk", bufs=1))
        pool_wv = top.enter_context(tc.tile_pool(name="wvp", bufs=1))

        # input DMAs split per tile so the first matmul only waits on xt +
        # wqk's first column block; the rest stream under compute.
        xt_t = pool_x.tile([P, NDT, N], bf, tag="xt")
        xt_r = xt_d[:].rearrange("(i p) n -> p i n", p=P)
        for i in range(NDT):
            nc.sync.dma_start(out=xt_t[:, i, :], in_=xt_r[:, i, :])
        wqk_t = pool_w.tile([P, NDT, 2 * D], bf, tag="wqk")
        wqk_r = wqk_d[:].rearrange("(i p) d -> p i d", p=P)
        # blocks 0 (q o=0) and 6 (k o=0) first: the serial prefix needs
        # them; wv next (v1 follows immediately); the rest stream under
        # compute.
        for o in (0, NDT):
            nc.sync.dma_start(out=wqk_t[:, :, o * P:(o + 1) * P],
                              in_=wqk_r[:, :, o * P:(o + 1) * P])
        load_ones(vaug1)
        wv_t = pool_wv.tile([P, NDT, D], bf, tag="wv")
        wv_r = wv_d[:].rearrange("(i p) d -> p i d", p=P)
        for i in range(NDT):
            nc.sync.dma_start(out=wv_t[:, i, :], in_=wv_r[:, i, :])
        for o in range(2 * NDT):
            if o not in (0, NDT):
                nc.sync.dma_start(out=wqk_t[:, :, o * P:(o + 1) * P],
                                  in_=wqk_r[:, :, o * P:(o + 1) * P])
        x2t_t = pool_x.tile([P, NDT, N], bf, tag="xt")
        x2t_r = x2t_d[:].rearrange("(i p) n -> p i n", p=P)
        for i in range(NDT):
            nc.sync.dma_start(out=x2t_t[:, i, :], in_=x2t_r[:, i, :])
        load_ones(vaug2)
        nc.sync.dma_start(
            out=wp_t, in_=wp_d[:].rearrange("(g p) d -> p g d", p=P))
        nc.sync.dma_start(out=bias_t, in_=bias_d[:])

        def qkv_T_o(xt_src, colblk, o, dst_sb, evict):
            """one output tile [128, N] of q/k-transposed formation."""
            ps = pp_s.tile([P, N], dt.float32, tag="S")
            for i in range(NDT):
                wt = wqk_t[:, i, colblk * D + o * P: colblk * D + (o + 1) * P]
                for c in range(2):
                    nc.tensor.matmul(
                        ps[:, c * 512:(c + 1) * 512],
                        wt,
                        xt_src[:, i, c * 512:(c + 1) * 512],
                        start=(i == 0), stop=(i == NDT - 1))
            evict(dst_sb[:, o, :], ps[:])

        def v_t(xt_src, vaug_t, t, evict):
            """one token tile [128, D] of v formation into vaug."""
            ps = pp_s.tile([P, D], dt.float32, tag="S")
            for i in range(NDT):
                for c0, cn in ((0, 512), (512, 256)):
                    nc.tensor.matmul(
                        ps[:, c0:c0 + cn],
                        xt_src[:, i, t * P:(t + 1) * P],
                        wv_t[:, i, c0:c0 + cn],
                        start=(i == 0), stop=(i == NDT - 1))
            dstv = vaug_t[:, t, :].rearrange(
                "p (h e) -> p h e", e=AUG)[:, :, 0:HD]
            evict(dstv, ps[:].rearrange("p (h e) -> p h e", e=HD))

        # phase A serial prefix: only what branch-0's first g needs —
        # q(o=0), k1(o=0), all of v1 (AV consumes every token tile in g=0).
        # ACT evictions (ACT is free here).
        qkv_T_o(xt_t, 0, 0, qT, nc.scalar.copy)
        qkv_T_o(xt_t, 1, 0, kT1, nc.scalar.copy)
        for t in range(NQT):
            v_t(xt_t, vaug1, t, nc.scalar.copy)

        # remaining QKV work rides branch-0 attention's PE slack (the g loop
        # is ACT-bound): q/k1 tile o is needed by g=o, so pairs go first in
        # order; k2/v2 must finish before branch-1 starts. DVE evictions
        # (ACT is saturated by exp there).
        thunks0 = []
        for o in range(1, NDT):
            thunks0.append(lambda o=o: qkv_T_o(xt_t, 0, o, qT,
                                               nc.vector.tensor_copy))
            thunks0.append(lambda o=o: qkv_T_o(xt_t, 1, o, kT1,
                                               nc.vector.tensor_copy))
        for o in range(NDT):
            thunks0.append(lambda o=o: qkv_T_o(x2t_t, 1, o, kT2,
                                               nc.vector.tensor_copy))
        for t in range(NQT):
            thunks0.append(lambda t=t: v_t(x2t_t, vaug2, t,
                                           nc.vector.tensor_copy))

        # ================= phase B: attention + proj ====================
        pool_osb = top.enter_context(tc.tile_pool(name="osb", bufs=4))
        pool_rc = top.enter_context(tc.tile_pool(name="rc", bufs=4))
        pool_rb = top.enter_context(tc.tile_pool(name="rb", bufs=2))
        pool_res = top.enter_context(tc.tile_pool(name="res", bufs=2))
        pool_otm = top.enter_context(tc.tile_pool(name="otm", bufs=2))

        def proj_qi(ot_t, br, qi):
            ps = pp_o.tile([P, D], dt.float32, tag="O")
            for g in range(NDT):
                for c0, cn in ((0, 512), (512, 256)):
                    nc.tensor.matmul(
                        ps[:, c0:c0 + cn],
                        ot_t[:, g, qi * P:(qi + 1) * P],
                        wp_t[:, g, c0:c0 + cn],
                        start=(g == 0), stop=(g == NDT - 1))
            res = pool_res.tile([P, D], dt.float32, tag="res")
            nc.vector.tensor_add(res[:], ps[:], bias_t[:])
            nc.sync.dma_start(out=out_d[br, qi * P:(qi + 1) * P, :],
                              in_=res[:])

        def attention(kT_t, vaug_t, ot, extra, budget):
            """head-pair ladder over g; `extra` thunks fill PE slack.
            `budget` = max extra thunks to pop per g iteration. The
            normalize tail of g is deferred one iteration so its DVE ops
            never sit in front of g+1's PSUM eviction in the FIFO (the rb
            broadcast's DMA latency is hidden by then)."""
            pending = []
            for g in range(NDT):
                po = [pp_o.tile([AUG, N], dt.float32, tag="O",
                                name=f"po{g}_{hh}") for hh in range(2)]

                def emit_av(kj):
                    for hh in range(2):
                        h = 2 * g + hh
                        for c in range(2):
                            nc.tensor.matmul(
                                po[hh][:, c * 512:(c + 1) * 512],
                                vaug_t[:, kj, h * AUG:(h + 1) * AUG],
                                pt2[:, hh, kj % KJR, c * 512:(c + 1) * 512],
                                start=(kj == 0), stop=(kj == NQT - 1),
                                skip_group_check=True)

                for kj in range(NQT):
                    for c in range(2):
                        ps = pp_s.tile([P, 2, 512], dt.float32, tag="S")
                        nc.tensor.matmul(
                            ps[:, 0, :],
                            kT_t[0:HD, g, kj * P:(kj + 1) * P],
                            qT[0:HD, g, c * 512:(c + 1) * 512],
                            start=True, stop=True)
                        nc.tensor.matmul(
                            ps[:, 1, :],
                            kT_t[HD:P, g, kj * P:(kj + 1) * P],
                            qT[HD:P, g, c * 512:(c + 1) * 512],
                            start=True, stop=True)
                        nc.scalar.activation(
                            pt2[:, :, kj % KJR, c * 512:(c + 1) * 512],
                            ps[:],
                            mybir.ActivationFunctionType.Exp, scale=SCALE)
                    if kj >= 1:
                        emit_av(kj - 1)
                emit_av(NQT - 1)

                # evict po fast (frees PSUM); the full normalize chain is
                # deferred so it never sits ahead of the next g's evicts
                osb = [pool_osb.tile([AUG, N], dt.float32, tag="osb",
                                     name=f"osb{g}_{hh}")
                       for hh in range(2)]
                for hh in range(2):
                    nc.vector.tensor_copy(osb[hh][:], po[hh][:])

                def normalize(g=g, osb=osb):
                    rdrm = dram_rb.tile([2, N], dt.float32, tag="rd")
                    for hh in range(2):
                        nc.sync.dma_start(out=rdrm[hh, :],
                                          in_=osb[hh][HD:HD + 1, :])
                    rcol = pool_rc.tile([P, 2, 8], dt.float32, tag="rc")
                    nc.sync.dma_start(
                        out=rcol,
                        in_=rdrm[:].rearrange("a (p c) -> p a c", p=P))
                    rcol2 = pool_rc.tile([P, 2, 8], dt.float32, tag="rc2")
                    nc.vector.reciprocal(rcol2[:], rcol[:])
                    rdrm2 = dram_rb.tile([2, N], dt.float32, tag="rd2")
                    nc.sync.dma_start(
                        out=rdrm2[:].rearrange("a (p c) -> p a c", p=P),
                        in_=rcol2)
                    rb = pool_rb.tile([HD, 2, N], dt.float32, tag="rb")
                    nc.sync.dma_start(
                        out=rb[:, 0, :],
                        in_=rdrm2[0, :].partition_broadcast(HD))
                    nc.sync.dma_start(
                        out=rb[:, 1, :],
                        in_=rdrm2[1, :].partition_broadcast(HD))
                    # DVE operands must share partitions 0:HD; hh=1's
                    # result is partition-shifted into ot[HD:] by DMA.
                    nc.vector.tensor_mul(ot[0:HD, g, :], osb[0][0:HD, :],
                                         rb[:, 0, :])
                    otm = pool_otm.tile([HD, N], bf, tag="otm")
                    nc.vector.tensor_mul(otm[:], osb[1][0:HD, :],
                                         rb[:, 1, :])
                    nc.sync.dma_start(out=ot[HD:P, g, :], in_=otm[:])

                pending.append(normalize)
                if len(pending) > 1:
                    pending.pop(0)()
                for _ in range(budget):
                    if extra:
                        extra.pop(0)()
            while extra:
                extra.pop(0)()
            while pending:
                pending.pop(0)()

        attention(kT1, vaug1, ot0, thunks0, budget=5)
        # 5 proj(br0) tiles ride branch-1; 3 are held back to keep the PE
        # warm through branch-1's final normalize chain.
        extra = [(lambda qi=qi: proj_qi(ot0, 0, qi)) for qi in range(NQT - 3)]
        attention(kT2, vaug2, ot1, extra, budget=1)
        for qi in range(NQT - 3, NQT):
            proj_qi(ot0, 0, qi)
        for qi in range(NQT):
            proj_qi(ot1, 1, qi)

    n = _split_multiwaits(nc)
    _built = (nc, n)
    return _built


def _host_prep(x, x2, qkv_w, proj_w, proj_b):
    """-> list of 8 per-core input maps (bf16 operands, f32 bias)."""
    import ml_dtypes
    b16 = lambda a: np.ascontiguousarray(a).astype(ml_dtypes.bfloat16)

    xt = np.transpose(np.asarray(x), (0, 2, 1))
    x2t = np.transpose(np.asarray(x2), (0, 2, 1))
    wqk = b16(np.asarray(qkv_w)[:2 * D].T)      # [768, 1536]
    wv = b16(np.asarray(qkv_w)[2 * D:].T)       # [768, 768]
    wp = b16(np.asarray(proj_w).T)              # [768, 768]
    bias = np.broadcast_to(np.asarray(proj_b, dtype=np.float32),
                           (P, D)).copy()
    ones = np.ones((P, NQT * H), dtype=ml_dtypes.bfloat16)
    maps = []
    for c in range(B):
        maps.append({
            "xt": b16(xt[c]), "x2t": b16(x2t[c]),
            "wqk": wqk, "wv": wv, "wp": wp, "bias": bias,
            "ones": ones,
        })
    return maps


def kernel(x, x2, qkv_w, proj_w, proj_b, trace=False, tmpdir=None):
    nc, _ = _build()
    from concourse.bass_utils import run_bass_kernel_spmd
    in_maps = _host_prep(x, x2, qkv_w, proj_w, proj_b)
    res = run_bass_kernel_spmd(nc, in_maps, list(range(B)), trace=trace,
                               tmpdir=tmpdir)
    kernel.last_exec_time_ns = res.exec_time_ns
    out = np.stack([res.results[c]["out"] for c in range(B)])  # [B,2,N,D]
    out1 = np.ascontiguousarray(out[:, 0])
    out2 = np.ascontiguousarray(out[:, 1])
    return (out1, out2)


kernel.last_exec_time_ns = None


# revision 29
# speedup vs baseline: 1.7051x; 1.0086x over previous
"""Two-branch attention kernel for Trainium2 (8 NeuronCores, batch-parallel).

out1 = proj(softmax(q k^T / 8) v),  out2 = proj(softmax(q k2^T / 8) v2)
with q,k,v from x and k2,v2 from x2 (q shared across branches).

Sharding: batch dim (8) -> one batch element per core. No collectives.

Design (per core, all matmul operands bf16, PSUM f32):
  QKV:  qT/kT [dim,tok] via W-stationary matmuls; v [tok,dim] via
        x-stationary matmuls into a ones-augmented buffer (AUG=65 column
        carries softmax row sums for free). k2/v2 kept in SBUF (no DRAM
        spill); their formation is interleaved into branch-1 attention so
        the PE fills the ACT-bound softmax gaps.
  Attn: per (kj,c): S^T chunk [128,2,512] in a double-buffered PSUM pair;
        the two head-half matmuls land on PE row-groups 0/64 and run
        concurrently. exp on ScalarE (scale=1/8, no max subtraction)
        writes P^T bf16; AV (V_aug stationary) pipelined one kj behind.
  Norm: po -> osb evict on DVE; row sums bounce DRAM -> [128,2,8] so the
        reciprocal uses all partitions (~0.1us, not 6.5us); broadcast back
        and multiply into ot (bf16).
  Proj: ot tiles stationary, stream W_proj^T, add bias on DVE, DMA out.
        proj(br0) interleaved into branch-2 attention.
"""
import sys
for _p in ('/opt/trn_rl_repo',):
    if _p not in sys.path:
        sys.path.insert(0, _p)

import numpy as np

MODE = 'bf16'

B, N, D, H, HD = 8, 1024, 768, 12, 64
SCALE = HD ** -0.5
NDT = D // 128       # 6 dim tiles
NQT = N // 128       # 8 token tiles
P = 128
AUG = HD + 1         # 65: head dim + ones column for row sums


# ----------------------------------------------------------------------------
# workaround: walrus rejects >2 sem waits on one instruction; TileContext's
# tail drain carries one wait per active logical proc. Split them across
# single-wait SP nops and emit a bare drain.
def _install_tilefix():
    import bass_rust
    import concourse.tile as tile

    def _drain_and_barrier_split(self, tick_clock, wait_clock):
        gc = tick_clock.global_clock
        ticks = [gc[i] for i in range(27)]
        for i, t in enumerate(ticks):
            if t > 0:
                vc = bass_rust.VectorClock(
                    [t if j == i else 0 for j in range(len(ticks))])
                nop = self.nc.sync.nop()
                wait_clock.add_sem_waits(
                    nop.ins, bass_rust.ScopedClock({None: vc}))
        self.nc.sync.drain()
        self.nc.all_engine_barrier()
        assert self.sems is not None
        popped = self.nc._tile_sem_poison_stack.pop()
        assert popped is self._sem_poison
        self.nc.clear_and_free_semaphores(list(self.sems.allocated().values()))
        self.nc.all_engine_barrier()

    tile.TileContext._drain_and_barrier = _drain_and_barrier_split


def _split_multiwaits(nc, max_waits=1):
    """walrus codegen rejects instructions carrying more than `max_waits`
    sync waits; hoist the extras onto same-engine nops placed just before."""
    import bass_rust
    import concourse.mybir as mybir
    cnt = 0
    for bb in nc.main_func.blocks:
        insts = bb.instructions
        i = 0
        while i < len(insts):
            ins = insts[i]
            si = getattr(ins, 'sync_info', None)
            if si is not None and si.on_wait and len(si.on_wait) > max_waits:
                waits = list(si.on_wait)
                extras, keep = waits[:-max_waits], waits[-max_waits:]
                for w in extras:
                    nop = mybir.InstNoOp(name=f"I-swx{cnt}", ins=[], outs=[])
                    cnt += 1
                    nop.engine = ins.engine
                    nop.sync_info = bass_rust.SyncInfo(on_wait=[w],
                                                       on_update=[])
                    insts.insert(i, nop)
                    i += 1
                ins.sync_info = bass_rust.SyncInfo(
                    on_wait=keep, on_update=list(si.on_update))
            i += 1
    return cnt


_built = None


def _build():
    """Build the SPMD bass program once. Returns (nc, n_split_waits)."""
    global _built
    if _built is not None:
        return _built
    _install_tilefix()
    from contextlib import ExitStack
    import concourse.bass as bass
    import concourse.tile as tile
    from concourse import mybir

    dt = mybir.dt
    bf = dt.bfloat16

    nc = bass.Bass("TRN2", target_bir_lowering=False, debug=False,
                   num_devices=8)

    # DRAM I/O (per core)
    xt_d = nc.dram_tensor("xt", [D, N], bf, kind="ExternalInput")
    x2t_d = nc.dram_tensor("x2t", [D, N], bf, kind="ExternalInput")
    wqk_d = nc.dram_tensor("wqk", [D, 2 * D], bf, kind="ExternalInput")
    wv_d = nc.dram_tensor("wv", [D, D], bf, kind="ExternalInput")
    wp_d = nc.dram_tensor("wp", [D, D], bf, kind="ExternalInput")
    bias_d = nc.dram_tensor("bias", [P, D], dt.float32, kind="ExternalInput")
    ones_d = nc.dram_tensor("ones", [P, NQT * H], bf, kind="ExternalInput")
    out_d = nc.dram_tensor("out", [2, N, D], dt.float32,
                           kind="ExternalOutput")

    with tile.TileContext(nc) as tc, ExitStack() as top:
        pp_s = top.enter_context(tc.tile_pool(name="ps_s", bufs=2,
                                              space="PSUM"))
        pp_o = top.enter_context(tc.tile_pool(name="ps_o", bufs=2,
                                              space="PSUM"))
        dram_rb = top.enter_context(tc.tile_pool(name="dram_rb", bufs=4,
                                                 space="DRAM"))
        persist = top.enter_context(tc.tile_pool(name="persist", bufs=1))

        # persistent SBUF tiles
        qT = persist.tile([P, NDT, N], bf, tag="qT")
        kT1 = persist.tile([P, NDT, N], bf, tag="kT1")
        kT2 = persist.tile([P, NDT, N], bf, tag="kT2")
        vaug1 = persist.tile([P, NQT, H * AUG], bf, tag="vaug1")
        vaug2 = persist.tile([P, NQT, H * AUG], bf, tag="vaug2")
        KJR = 4          # pt2 ring depth over kj (AV trails exp by <=2)
        pt2 = persist.tile([P, 2, KJR, N], bf, tag="pt2")
        ot0 = persist.tile([P, NDT, N], bf, tag="ot0")
        ot1 = persist.tile([P, NDT, N], bf, tag="ot1")
        wp_t = persist.tile([P, NDT, D], bf, tag="wp")
        bias_t = persist.tile([P, D], dt.float32, tag="bias")

        def load_ones(vaug_t):
            nc.sync.dma_start(
                out=vaug_t[:].rearrange("p t (h e) -> p t h e",
                                        e=AUG)[:, :, :, HD:AUG],
                in_=ones_d[:].rearrange("p (t h e) -> p t h e", h=H, e=1))

        # ================= phase A pools (kept open through br0) =========
        pool_x = top.enter_context(tc.tile_pool(name="xa", bufs=2))
        pool_w = top.enter_context(tc.tile_pool(name="wqk", bufs=1))
        pool_wv = top.enter_context(tc.tile_pool(name="wvp", bufs=1))

        # input DMAs split per tile so the first matmul only waits on xt +
        # wqk's first column block; the rest stream under compute.
        xt_t = pool_x.tile([P, NDT, N], bf, tag="xt")
        xt_r = xt_d[:].rearrange("(i p) n -> p i n", p=P)
        for i in range(NDT):
            nc.sync.dma_start(out=xt_t[:, i, :], in_=xt_r[:, i, :])
        wqk_t = pool_w.tile([P, NDT, 2 * D], bf, tag="wqk")
        wqk_r = wqk_d[:].rearrange("(i p) d -> p i d", p=P)
        # blocks 0 (q o=0) and 6 (k o=0) first: the serial prefix needs
        # them; wv next (v1 follows immediately); the rest stream under
        # compute.
        for o in (0, NDT):
            nc.sync.dma_start(out=wqk_t[:, :, o * P:(o + 1) * P],
                              in_=wqk_r[:, :, o * P:(o + 1) * P])
        load_ones(vaug1)
        wv_t = pool_wv.tile([P, NDT, D], bf, tag="wv")
        wv_r = wv_d[:].rearrange("(i p) d -> p i d", p=P)
        for i in range(NDT):
            nc.sync.dma_start(out=wv_t[:, i, :], in_=wv_r[:, i, :])
        for o in range(2 * NDT):
            if o not in (0, NDT):
                nc.sync.dma_start(out=wqk_t[:, :, o * P:(o + 1) * P],
                                  in_=wqk_r[:, :, o * P:(o + 1) * P])
        x2t_t = pool_x.tile([P, NDT, N], bf, tag="xt")
        x2t_r = x2t_d[:].rearrange("(i p) n -> p i n", p=P)
        for i in range(NDT):
            nc.sync.dma_start(out=x2t_t[:, i, :], in_=x2t_r[:, i, :])
        load_ones(vaug2)
        nc.sync.dma_start(
            out=wp_t, in_=wp_d[:].rearrange("(g p) d -> p g d", p=P))
        nc.sync.dma_start(out=bias_t, in_=bias_d[:])

        def qkv_T_o(xt_src, colblk, o, dst_sb, evict):
            """one output tile [128, N] of q/k-transposed formation."""
            ps = pp_s.tile([P, N], dt.float32, tag="S")
            for i in range(NDT):
                wt = wqk_t[:, i, colblk * D + o * P: colblk * D + (o + 1) * P]
                for c in range(2):
                    nc.tensor.matmul(
                        ps[:, c * 512:(c + 1) * 512],
                        wt,
                        xt_src[:, i, c * 512:(c + 1) * 512],
                        start=(i == 0), stop=(i == NDT - 1))
            evict(dst_sb[:, o, :], ps[:])

        def v_t(xt_src, vaug_t, t, evict):
            """one token tile [128, D] of v formation into vaug."""
            ps = pp_s.tile([P, D], dt.float32, tag="S")
            for i in range(NDT):
                for c0, cn in ((0, 512), (512, 256)):
                    nc.tensor.matmul(
                        ps[:, c0:c0 + cn],
                        xt_src[:, i, t * P:(t + 1) * P],
                        wv_t[:, i, c0:c0 + cn],
                        start=(i == 0), stop=(i == NDT - 1))
            dstv = vaug_t[:, t, :].rearrange(
                "p (h e) -> p h e", e=AUG)[:, :, 0:HD]
            evict(dstv, ps[:].rearrange("p (h e) -> p h e", e=HD))

        # phase A serial prefix: only what branch-0's very first AV needs —
        # q(o=0), k1(o=0), v1(t=0). v1(t) for t>=1 threads into g=0's kj
        # loop (AV runs one kj behind, so v1(t) lands just in time).
        # ACT evictions (ACT is free here).
        qkv_T_o(xt_t, 0, 0, qT, nc.scalar.copy)
        qkv_T_o(xt_t, 1, 0, kT1, nc.scalar.copy)
        v_t(xt_t, vaug1, 0, nc.scalar.copy)
        kj_extra = [(lambda t=t: v_t(xt_t, vaug1, t, nc.vector.tensor_copy))
                    for t in range(1, NQT)]

        # remaining QKV work rides branch-0 attention's PE slack (the g loop
        # is ACT-bound): q/k1 tile o is needed by g=o, so pairs go first in
        # order; k2/v2 must finish before branch-1 starts. DVE evictions
        # (ACT is saturated by exp there).
        thunks0 = []
        for o in range(1, NDT):
            thunks0.append(lambda o=o: qkv_T_o(xt_t, 0, o, qT,
                                               nc.vector.tensor_copy))
            thunks0.append(lambda o=o: qkv_T_o(xt_t, 1, o, kT1,
                                               nc.vector.tensor_copy))
        for o in range(NDT):
            thunks0.append(lambda o=o: qkv_T_o(x2t_t, 1, o, kT2,
                                               nc.vector.tensor_copy))
        for t in range(NQT):
            thunks0.append(lambda t=t: v_t(x2t_t, vaug2, t,
                                           nc.vector.tensor_copy))

        # ================= phase B: attention + proj ====================
        pool_osb = top.enter_context(tc.tile_pool(name="osb", bufs=4))
        pool_rc = top.enter_context(tc.tile_pool(name="rc", bufs=4))
        pool_rb = top.enter_context(tc.tile_pool(name="rb", bufs=2))
        pool_res = top.enter_context(tc.tile_pool(name="res", bufs=2))
        pool_otm = top.enter_context(tc.tile_pool(name="otm", bufs=2))

        def proj_qi(ot_t, br, qi):
            ps = pp_o.tile([P, D], dt.float32, tag="O")
            for g in range(NDT):
                for c0, cn in ((0, 512), (512, 256)):
                    nc.tensor.matmul(
                        ps[:, c0:c0 + cn],
                        ot_t[:, g, qi * P:(qi + 1) * P],
                        wp_t[:, g, c0:c0 + cn],
                        start=(g == 0), stop=(g == NDT - 1))
            res = pool_res.tile([P, D], dt.float32, tag="res")
            nc.vector.tensor_add(res[:], ps[:], bias_t[:])
            nc.sync.dma_start(out=out_d[br, qi * P:(qi + 1) * P, :],
                              in_=res[:])

        def attention(kT_t, vaug_t, ot, extra, budget, kj_extra=()):
            """head-pair ladder over g; `extra` thunks fill PE slack.
            `budget` = max extra thunks to pop per g iteration. The
            normalize tail of g is deferred one iteration so its DVE ops
            never sit in front of g+1's PSUM eviction in the FIFO (the rb
            broadcast's DMA latency is hidden by then)."""
            pending = []
            for g in range(NDT):
                po = [pp_o.tile([AUG, N], dt.float32, tag="O",
                                name=f"po{g}_{hh}") for hh in range(2)]

                def emit_av(kj):
                    for hh in range(2):
                        h = 2 * g + hh
                        for c in range(2):
                            nc.tensor.matmul(
                                po[hh][:, c * 512:(c + 1) * 512],
                                vaug_t[:, kj, h * AUG:(h + 1) * AUG],
                                pt2[:, hh, kj % KJR, c * 512:(c + 1) * 512],
                                start=(kj == 0), stop=(kj == NQT - 1),
                                skip_group_check=True)

                for kj in range(NQT):
                    for c in range(2):
                        ps = pp_s.tile([P, 2, 512], dt.float32, tag="S")
                        nc.tensor.matmul(
                            ps[:, 0, :],
                            kT_t[0:HD, g, kj * P:(kj + 1) * P],
                            qT[0:HD, g, c * 512:(c + 1) * 512],
                            start=True, stop=True)
                        nc.tensor.matmul(
                            ps[:, 1, :],
                            kT_t[HD:P, g, kj * P:(kj + 1) * P],
                            qT[HD:P, g, c * 512:(c + 1) * 512],
                            start=True, stop=True)
                        nc.scalar.activation(
                            pt2[:, :, kj % KJR, c * 512:(c + 1) * 512],
                            ps[:],
                            mybir.ActivationFunctionType.Exp, scale=SCALE)
                    if kj_extra:
                        kj_extra.pop(0)()
                    if kj >= 1:
                        emit_av(kj - 1)
                emit_av(NQT - 1)

                # evict po fast (frees PSUM); the full normalize chain is
                # deferred so it never sits ahead of the next g's evicts
                osb = [pool_osb.tile([AUG, N], dt.float32, tag="osb",
                                     name=f"osb{g}_{hh}")
                       for hh in range(2)]
                for hh in range(2):
                    nc.vector.tensor_copy(osb[hh][:], po[hh][:])

                def normalize(g=g, osb=osb):
                    rdrm = dram_rb.tile([2, N], dt.float32, tag="rd")
                    for hh in range(2):
                        nc.sync.dma_start(out=rdrm[hh, :],
                                          in_=osb[hh][HD:HD + 1, :])
                    rcol = pool_rc.tile([P, 2, 8], dt.float32, tag="rc")
                    nc.sync.dma_start(
                        out=rcol,
                        in_=rdrm[:].rearrange("a (p c) -> p a c", p=P))
                    rcol2 = pool_rc.tile([P, 2, 8], dt.float32, tag="rc2")
                    nc.vector.reciprocal(rcol2[:], rcol[:])
                    rdrm2 = dram_rb.tile([2, N], dt.float32, tag="rd2")
                    nc.sync.dma_start(
                        out=rdrm2[:].rearrange("a (p c) -> p a c", p=P),
                        in_=rcol2)
                    rb = pool_rb.tile([HD, 2, N], dt.float32, tag="rb")
                    nc.sync.dma_start(
                        out=rb[:, 0, :],
                        in_=rdrm2[0, :].partition_broadcast(HD))
                    nc.sync.dma_start(
                        out=rb[:, 1, :],
                        in_=rdrm2[1, :].partition_broadcast(HD))
                    # DVE operands must share partitions 0:HD; hh=1's
                    # result is partition-shifted into ot[HD:] by DMA.
                    nc.vector.tensor_mul(ot[0:HD, g, :], osb[0][0:HD, :],
                                         rb[:, 0, :])
                    otm = pool_otm.tile([HD, N], bf, tag="otm")
                    nc.vector.tensor_mul(otm[:], osb[1][0:HD, :],
                                         rb[:, 1, :])
                    nc.sync.dma_start(out=ot[HD:P, g, :], in_=otm[:])

                pending.append(normalize)
                if len(pending) > 1:
                    pending.pop(0)()
                for _ in range(budget):
                    if extra:
                        extra.pop(0)()
            while extra:
                extra.pop(0)()
            while pending:
                pending.pop(0)()

        attention(kT1, vaug1, ot0, thunks0, budget=5, kj_extra=kj_extra)
        # all 8 proj(br0) tiles go through `extra`: budget=1 pops 6 in the
        # g loop, and the final 2 drain BEFORE the deferred normalize flush
        # so their PE work isn't sequenced behind branch-1's last chain.
        extra = [(lambda qi=qi: proj_qi(ot0, 0, qi)) for qi in range(NQT)]
        attention(kT2, vaug2, ot1, extra, budget=1)
        for qi in range(NQT):
            proj_qi(ot1, 1, qi)

    n = _split_multiwaits(nc)
    _built = (nc, n)
    return _built


def _host_prep(x, x2, qkv_w, proj_w, proj_b):
    """-> list of 8 per-core input maps (bf16 operands, f32 bias)."""
    import ml_dtypes
    b16 = lambda a: np.ascontiguousarray(a).astype(ml_dtypes.bfloat16)

    xt = np.transpose(np.asarray(x), (0, 2, 1))
    x2t = np.transpose(np.asarray(x2), (0, 2, 1))
    wqk = b16(np.asarray(qkv_w)[:2 * D].T)      # [768, 1536]
    wv = b16(np.asarray(qkv_w)[2 * D:].T)       # [768, 768]
    wp = b16(np.asarray(proj_w).T)              # [768, 768]
    bias = np.broadcast_to(np.asarray(proj_b, dtype=np.float32),
                           (P, D)).copy()
    ones = np.ones((P, NQT * H), dtype=ml_dtypes.bfloat16)
    maps = []
    for c in range(B):
        maps.append({
            "xt": b16(xt[c]), "x2t": b16(x2t[c]),
            "wqk": wqk, "wv": wv, "wp": wp, "bias": bias,
            "ones": ones,
        })
    return maps


def kernel(x, x2, qkv_w, proj_w, proj_b, trace=False, tmpdir=None):
    nc, _ = _build()
    from concourse.bass_utils import run_bass_kernel_spmd
    in_maps = _host_prep(x, x2, qkv_w, proj_w, proj_b)
    res = run_bass_kernel_spmd(nc, in_maps, list(range(B)), trace=trace,
                               tmpdir=tmpdir)
    kernel.last_exec_time_ns = res.exec_time_ns
    out = np.stack([res.results[c]["out"] for c in range(B)])  # [B,2,N,D]
    out1 = np.ascontiguousarray(out[:, 0])
    out2 = np.ascontiguousarray(out[:, 1])
    return (out1, out2)


kernel.last_exec_time_ns = None


# revision 34
# speedup vs baseline: 1.7067x; 1.0009x over previous
"""Two-branch attention kernel for Trainium2 (8 NeuronCores, batch-parallel).

out1 = proj(softmax(q k^T / 8) v),  out2 = proj(softmax(q k2^T / 8) v2)
with q,k,v from x and k2,v2 from x2 (q shared across branches).

Sharding: batch dim (8) -> one batch element per core. No collectives.

Design (per core, all matmul operands bf16, PSUM f32):
  QKV:  qT/kT [dim,tok] via W-stationary matmuls; v [tok,dim] via
        x-stationary matmuls into a ones-augmented buffer (AUG=65 column
        carries softmax row sums for free). k2/v2 kept in SBUF (no DRAM
        spill); their formation is interleaved into branch-1 attention so
        the PE fills the ACT-bound softmax gaps.
  Attn: per (kj,c): S^T chunk [128,2,512] in a double-buffered PSUM pair;
        the two head-half matmuls land on PE row-groups 0/64 and run
        concurrently. exp on ScalarE (scale=1/8, no max subtraction)
        writes P^T bf16; AV (V_aug stationary) pipelined one kj behind.
  Norm: po -> osb evict on DVE; row sums bounce DRAM -> [128,2,8] so the
        reciprocal uses all partitions (~0.1us, not 6.5us); broadcast back
        and multiply into ot (bf16).
  Proj: ot tiles stationary, stream W_proj^T, add bias on DVE, DMA out.
        proj(br0) interleaved into branch-2 attention.
"""
import sys
for _p in ('/opt/trn_rl_repo',):
    if _p not in sys.path:
        sys.path.insert(0, _p)

import numpy as np

MODE = 'bf16'

B, N, D, H, HD = 8, 1024, 768, 12, 64
SCALE = HD ** -0.5
NDT = D // 128       # 6 dim tiles
NQT = N // 128       # 8 token tiles
P = 128
AUG = HD + 1         # 65: head dim + ones column for row sums


# ----------------------------------------------------------------------------
# workaround: walrus rejects >2 sem waits on one instruction; TileContext's
# tail drain carries one wait per active logical proc. Split them across
# single-wait SP nops and emit a bare drain.
def _install_tilefix():
    import bass_rust
    import concourse.tile as tile

    def _drain_and_barrier_split(self, tick_clock, wait_clock):
        gc = tick_clock.global_clock
        ticks = [gc[i] for i in range(27)]
        for i, t in enumerate(ticks):
            if t > 0:
                vc = bass_rust.VectorClock(
                    [t if j == i else 0 for j in range(len(ticks))])
                nop = self.nc.sync.nop()
                wait_clock.add_sem_waits(
                    nop.ins, bass_rust.ScopedClock({None: vc}))
        self.nc.sync.drain()
        self.nc.all_engine_barrier()
        assert self.sems is not None
        popped = self.nc._tile_sem_poison_stack.pop()
        assert popped is self._sem_poison
        self.nc.clear_and_free_semaphores(list(self.sems.allocated().values()))
        self.nc.all_engine_barrier()

    tile.TileContext._drain_and_barrier = _drain_and_barrier_split


def _split_multiwaits(nc, max_waits=1):
    """walrus codegen rejects instructions carrying more than `max_waits`
    sync waits; hoist the extras onto same-engine nops placed just before."""
    import bass_rust
    import concourse.mybir as mybir
    cnt = 0
    for bb in nc.main_func.blocks:
        insts = bb.instructions
        i = 0
        while i < len(insts):
            ins = insts[i]
            si = getattr(ins, 'sync_info', None)
            if si is not None and si.on_wait and len(si.on_wait) > max_waits:
                waits = list(si.on_wait)
                extras, keep = waits[:-max_waits], waits[-max_waits:]
                for w in extras:
                    nop = mybir.InstNoOp(name=f"I-swx{cnt}", ins=[], outs=[])
                    cnt += 1
                    nop.engine = ins.engine
                    nop.sync_info = bass_rust.SyncInfo(on_wait=[w],
                                                       on_update=[])
                    insts.insert(i, nop)
                    i += 1
                ins.sync_info = bass_rust.SyncInfo(
                    on_wait=keep, on_update=list(si.on_update))
            i += 1
    return cnt


def _enable_ldw_opt():
    """walrus's LDWEIGHTS optimization pass is disabled by default in this
    stack; our streams reload an unchanged stationary on ~1/3 of matmuls,
    which the pass can elide. Rewrite the flag at the run_command boundary."""
    import concourse.bass_utils as bu
    if getattr(bu, '_ldw_opt_patched', False):
        return
    orig = bu.run_command

    def run_command_ldw(cmd, *a, **kw):
        cmd = [c.replace('--enable-ldw-opt=false', '--enable-ldw-opt=true')
               if isinstance(c, str) else c for c in cmd]
        return orig(cmd, *a, **kw)

    bu.run_command = run_command_ldw
    bu._ldw_opt_patched = True


_built = None


def _build():
    """Build the SPMD bass program once. Returns (nc, n_split_waits)."""
    global _built
    if _built is not None:
        return _built
    _install_tilefix()
    from contextlib import ExitStack
    import concourse.bass as bass
    import concourse.tile as tile
    from concourse import mybir

    dt = mybir.dt
    bf = dt.bfloat16

    nc = bass.Bass("TRN2", target_bir_lowering=False, debug=False,
                   num_devices=8)

    # DRAM I/O (per core)
    xt_d = nc.dram_tensor("xt", [D, N], bf, kind="ExternalInput")
    x2t_d = nc.dram_tensor("x2t", [D, N], bf, kind="ExternalInput")
    wqk_d = nc.dram_tensor("wqk", [D, 2 * D], bf, kind="ExternalInput")
    wv_d = nc.dram_tensor("wv", [D, D], bf, kind="ExternalInput")
    wp_d = nc.dram_tensor("wp", [D, D], bf, kind="ExternalInput")
    bias_d = nc.dram_tensor("bias", [P, D], dt.float32, kind="ExternalInput")
    ones_d = nc.dram_tensor("ones", [P, NQT * H], bf, kind="ExternalInput")
    out_d = nc.dram_tensor("out", [2, N, D], dt.float32,
                           kind="ExternalOutput")

    with tile.TileContext(nc) as tc, ExitStack() as top:
        pp_s = top.enter_context(tc.tile_pool(name="ps_s", bufs=2,
                                              space="PSUM"))
        pp_o = top.enter_context(tc.tile_pool(name="ps_o", bufs=2,
                                              space="PSUM"))
        dram_rb = top.enter_context(tc.tile_pool(name="dram_rb", bufs=4,
                                                 space="DRAM"))
        persist = top.enter_context(tc.tile_pool(name="persist", bufs=1))

        # persistent SBUF tiles
        qT = persist.tile([P, NDT, N], bf, tag="qT")
        kT1 = persist.tile([P, NDT, N], bf, tag="kT1")
        kT2 = persist.tile([P, NDT, N], bf, tag="kT2")
        vaug1 = persist.tile([P, NQT, H * AUG], bf, tag="vaug1")
        vaug2 = persist.tile([P, NQT, H * AUG], bf, tag="vaug2")
        KJR = 4          # pt2 ring depth over kj (AV trails exp by <=2)
        pt2 = persist.tile([P, 2, KJR, N], bf, tag="pt2")
        ot0 = persist.tile([P, NDT, N], bf, tag="ot0")
        ot1 = persist.tile([P, NDT, N], bf, tag="ot1")
        wp_t = persist.tile([P, NDT, D], bf, tag="wp")
        bias_t = persist.tile([P, D], dt.float32, tag="bias")

        def load_ones(vaug_t):
            nc.sync.dma_start(
                out=vaug_t[:].rearrange("p t (h e) -> p t h e",
                                        e=AUG)[:, :, :, HD:AUG],
                in_=ones_d[:].rearrange("p (t h e) -> p t h e", h=H, e=1))

        # ================= phase A pools (kept open through br0) =========
        pool_x = top.enter_context(tc.tile_pool(name="xa", bufs=2))
        pool_w = top.enter_context(tc.tile_pool(name="wqk", bufs=1))
        pool_wv = top.enter_context(tc.tile_pool(name="wvp", bufs=1))

        # input DMAs split per tile so the first matmul only waits on xt +
        # wqk's first column block; the rest stream under compute.
        xt_t = pool_x.tile([P, NDT, N], bf, tag="xt")
        xt_r = xt_d[:].rearrange("(i p) n -> p i n", p=P)
        for i in range(NDT):
            nc.sync.dma_start(out=xt_t[:, i, :], in_=xt_r[:, i, :])
        wqk_t = pool_w.tile([P, NDT, 2 * D], bf, tag="wqk")
        wqk_r = wqk_d[:].rearrange("(i p) d -> p i d", p=P)
        # blocks 0 (q o=0) and 6 (k o=0) first: the serial prefix needs
        # them; wv next (v1 follows immediately); the rest stream under
        # compute.
        for o in (0, NDT):
            nc.sync.dma_start(out=wqk_t[:, :, o * P:(o + 1) * P],
                              in_=wqk_r[:, :, o * P:(o + 1) * P])
        load_ones(vaug1)
        wv_t = pool_wv.tile([P, NDT, D], bf, tag="wv")
        wv_r = wv_d[:].rearrange("(i p) d -> p i d", p=P)
        for i in range(NDT):
            nc.sync.dma_start(out=wv_t[:, i, :], in_=wv_r[:, i, :])
        for o in range(2 * NDT):
            if o not in (0, NDT):
                nc.sync.dma_start(out=wqk_t[:, :, o * P:(o + 1) * P],
                                  in_=wqk_r[:, :, o * P:(o + 1) * P])
        x2t_t = pool_x.tile([P, NDT, N], bf, tag="xt")
        x2t_r = x2t_d[:].rearrange("(i p) n -> p i n", p=P)
        for i in range(NDT):
            nc.sync.dma_start(out=x2t_t[:, i, :], in_=x2t_r[:, i, :])
        load_ones(vaug2)
        nc.sync.dma_start(
            out=wp_t, in_=wp_d[:].rearrange("(g p) d -> p g d", p=P))
        nc.sync.dma_start(out=bias_t, in_=bias_d[:])

        def qkv_T_o(xt_src, colblk, o, dst_sb, evict):
            """one output tile [128, N] of q/k-transposed formation."""
            ps = pp_s.tile([P, N], dt.float32, tag="S")
            for i in range(NDT):
                wt = wqk_t[:, i, colblk * D + o * P: colblk * D + (o + 1) * P]
                for c in range(2):
                    nc.tensor.matmul(
                        ps[:, c * 512:(c + 1) * 512],
                        wt,
                        xt_src[:, i, c * 512:(c + 1) * 512],
                        start=(i == 0), stop=(i == NDT - 1))
            evict(dst_sb[:, o, :], ps[:])

        def v_t(xt_src, vaug_t, t, evict):
            """one token tile [128, D] of v formation into vaug."""
            ps = pp_s.tile([P, D], dt.float32, tag="S")
            for i in range(NDT):
                for c0, cn in ((0, 512), (512, 256)):
                    nc.tensor.matmul(
                        ps[:, c0:c0 + cn],
                        xt_src[:, i, t * P:(t + 1) * P],
                        wv_t[:, i, c0:c0 + cn],
                        start=(i == 0), stop=(i == NDT - 1))
            dstv = vaug_t[:, t, :].rearrange(
                "p (h e) -> p h e", e=AUG)[:, :, 0:HD]
            evict(dstv, ps[:].rearrange("p (h e) -> p h e", e=HD))

        # phase A serial prefix: only what branch-0's very first AV needs —
        # q(o=0), k1(o=0), v1(t=0). v1(t) for t>=1 threads into g=0's kj
        # loop (AV runs one kj behind, so v1(t) lands just in time).
        # ACT evictions (ACT is free here).
        qkv_T_o(xt_t, 0, 0, qT, nc.scalar.copy)
        qkv_T_o(xt_t, 1, 0, kT1, nc.scalar.copy)
        v_t(xt_t, vaug1, 0, nc.scalar.copy)
        kj_extra = [(lambda t=t: v_t(xt_t, vaug1, t, nc.vector.tensor_copy))
                    for t in range(1, NQT)]

        # remaining QKV work rides branch-0 attention's PE slack (the g loop
        # is ACT-bound): q/k1 tile o is needed by g=o, so pairs go first in
        # order; k2/v2 must finish before branch-1 starts. DVE evictions
        # (ACT is saturated by exp there).
        thunks0 = []
        for o in range(1, NDT):
            thunks0.append(lambda o=o: qkv_T_o(xt_t, 0, o, qT,
                                               nc.vector.tensor_copy))
            thunks0.append(lambda o=o: qkv_T_o(xt_t, 1, o, kT1,
                                               nc.vector.tensor_copy))
        for o in range(NDT):
            thunks0.append(lambda o=o: qkv_T_o(x2t_t, 1, o, kT2,
                                               nc.vector.tensor_copy))
        for t in range(NQT):
            thunks0.append(lambda t=t: v_t(x2t_t, vaug2, t,
                                           nc.vector.tensor_copy))

        # ================= phase B: attention + proj ====================
        pool_osb = top.enter_context(tc.tile_pool(name="osb", bufs=4))
        pool_rc = top.enter_context(tc.tile_pool(name="rc", bufs=4))
        pool_rb = top.enter_context(tc.tile_pool(name="rb", bufs=2))
        pool_res = top.enter_context(tc.tile_pool(name="res", bufs=2))
        pool_otm = top.enter_context(tc.tile_pool(name="otm", bufs=2))

        def proj_qi(ot_t, br, qi):
            ps = pp_o.tile([P, D], dt.float32, tag="O")
            for g in range(NDT):
                for c0, cn in ((0, 512), (512, 256)):
                    nc.tensor.matmul(
                        ps[:, c0:c0 + cn],
                        ot_t[:, g, qi * P:(qi + 1) * P],
                        wp_t[:, g, c0:c0 + cn],
                        start=(g == 0), stop=(g == NDT - 1))
            res = pool_res.tile([P, D], dt.float32, tag="res")
            nc.vector.tensor_add(res[:], ps[:], bias_t[:])
            nc.sync.dma_start(out=out_d[br, qi * P:(qi + 1) * P, :],
                              in_=res[:])

        def attention(kT_t, vaug_t, ot, extra, budget, kj_extra=()):
            """head-pair ladder over g; `extra` thunks fill PE slack.
            `budget` = max extra thunks to pop per g iteration. The
            normalize tail of g is deferred one iteration so its DVE ops
            never sit in front of g+1's PSUM eviction in the FIFO (the rb
            broadcast's DMA latency is hidden by then)."""
            pending = []
            for g in range(NDT):
                po = [pp_o.tile([AUG, N], dt.float32, tag="O",
                                name=f"po{g}_{hh}") for hh in range(2)]

                def emit_av(kj):
                    for hh in range(2):
                        h = 2 * g + hh
                        for c in range(2):
                            nc.tensor.matmul(
                                po[hh][:, c * 512:(c + 1) * 512],
                                vaug_t[:, kj, h * AUG:(h + 1) * AUG],
                                pt2[:, hh, kj % KJR, c * 512:(c + 1) * 512],
                                start=(kj == 0), stop=(kj == NQT - 1),
                                skip_group_check=True)

                for kj in range(NQT):
                    for c in range(2):
                        ps = pp_s.tile([P, 2, 512], dt.float32, tag="S")
                        nc.tensor.matmul(
                            ps[:, 0, :],
                            kT_t[0:HD, g, kj * P:(kj + 1) * P],
                            qT[0:HD, g, c * 512:(c + 1) * 512],
                            start=True, stop=True)
                        nc.tensor.matmul(
                            ps[:, 1, :],
                            kT_t[HD:P, g, kj * P:(kj + 1) * P],
                            qT[HD:P, g, c * 512:(c + 1) * 512],
                            start=True, stop=True)
                        nc.scalar.activation(
                            pt2[:, :, kj % KJR, c * 512:(c + 1) * 512],
                            ps[:],
                            mybir.ActivationFunctionType.Exp, scale=SCALE)
                    if kj_extra:
                        kj_extra.pop(0)()
                    if kj >= 1:
                        emit_av(kj - 1)
                emit_av(NQT - 1)

                # evict po fast (frees PSUM); the full normalize chain is
                # deferred so it never sits ahead of the next g's evicts.
                # Last g: hh1 evict runs on the now-idle ACT so the tail
                # chain starts ~1.2us earlier.
                last = (g == NDT - 1)
                osb = [pool_osb.tile([AUG, N], dt.float32, tag="osb",
                                     name=f"osb{g}_{hh}")
                       for hh in range(2)]
                nc.vector.tensor_copy(osb[0][:], po[0][:])
                (nc.scalar.copy if last else nc.vector.tensor_copy)(
                    osb[1][:], po[1][:])

                def normalize(g=g, osb=osb):
                    rdrm = dram_rb.tile([2, N], dt.float32, tag="rd")
                    for hh in range(2):
                        nc.sync.dma_start(out=rdrm[hh, :],
                                          in_=osb[hh][HD:HD + 1, :])
                    rcol = pool_rc.tile([P, 2, 8], dt.float32, tag="rc")
                    nc.sync.dma_start(
                        out=rcol,
                        in_=rdrm[:].rearrange("a (p c) -> p a c", p=P))
                    rcol2 = pool_rc.tile([P, 2, 8], dt.float32, tag="rc2")
                    nc.vector.reciprocal(rcol2[:], rcol[:])
                    rdrm2 = dram_rb.tile([2, N], dt.float32, tag="rd2")
                    nc.sync.dma_start(
                        out=rdrm2[:].rearrange("a (p c) -> p a c", p=P),
                        in_=rcol2)
                    rb = pool_rb.tile([HD, 2, N], dt.float32, tag="rb")
                    nc.sync.dma_start(
                        out=rb[:, 0, :],
                        in_=rdrm2[0, :].partition_broadcast(HD))
                    nc.sync.dma_start(
                        out=rb[:, 1, :],
                        in_=rdrm2[1, :].partition_broadcast(HD))
                    # DVE operands must share partitions 0:HD; hh=1's
                    # result is partition-shifted into ot[HD:] by DMA.
                    nc.vector.tensor_mul(ot[0:HD, g, :], osb[0][0:HD, :],
                                         rb[:, 0, :])
                    otm = pool_otm.tile([HD, N], bf, tag="otm")
                    nc.vector.tensor_mul(otm[:], osb[1][0:HD, :],
                                         rb[:, 1, :])
                    nc.sync.dma_start(out=ot[HD:P, g, :], in_=otm[:])

                pending.append(normalize)
                if len(pending) > 1:
                    pending.pop(0)()
                for _ in range(budget):
                    if extra:
                        extra.pop(0)()
            while extra:
                extra.pop(0)()
            while pending:
                pending.pop(0)()

        attention(kT1, vaug1, ot0, thunks0, budget=5, kj_extra=kj_extra)
        # all 8 proj(br0) tiles go through `extra`: budget=1 pops 6 in the
        # g loop, and the final 2 drain BEFORE the deferred normalize flush
        # so their PE work isn't sequenced behind branch-1's last chain.
        extra = [(lambda qi=qi: proj_qi(ot0, 0, qi)) for qi in range(NQT)]
        attention(kT2, vaug2, ot1, extra, budget=1)
        for qi in range(NQT):
            proj_qi(ot1, 1, qi)

    n = _split_multiwaits(nc)
    _built = (nc, n)
    return _built


def _host_prep(x, x2, qkv_w, proj_w, proj_b):
    """-> list of 8 per-core input maps (bf16 operands, f32 bias)."""
    import ml_dtypes
    b16 = lambda a: np.ascontiguousarray(a).astype(ml_dtypes.bfloat16)

    xt = np.transpose(np.asarray(x), (0, 2, 1))
    x2t = np.transpose(np.asarray(x2), (0, 2, 1))
    wqk = b16(np.asarray(qkv_w)[:2 * D].T)      # [768, 1536]
    wv = b16(np.asarray(qkv_w)[2 * D:].T)       # [768, 768]
    wp = b16(np.asarray(proj_w).T)              # [768, 768]
    bias = np.broadcast_to(np.asarray(proj_b, dtype=np.float32),
                           (P, D)).copy()
    ones = np.ones((P, NQT * H), dtype=ml_dtypes.bfloat16)
    maps = []
    for c in range(B):
        maps.append({
            "xt": b16(xt[c]), "x2t": b16(x2t[c]),
            "wqk": wqk, "wv": wv, "wp": wp, "bias": bias,
            "ones": ones,
        })
    return maps


def kernel(x, x2, qkv_w, proj_w, proj_b, trace=False, tmpdir=None):
    nc, _ = _build()
    from concourse.bass_utils import run_bass_kernel_spmd
    in_maps = _host_prep(x, x2, qkv_w, proj_w, proj_b)
    res = run_bass_kernel_spmd(nc, in_maps, list(range(B)), trace=trace,
                               tmpdir=tmpdir)
    kernel.last_exec_time_ns = res.exec_time_ns
    out = np.stack([res.results[c]["out"] for c in range(B)])  # [B,2,N,D]
    out1 = np.ascontiguousarray(out[:, 0])
    out2 = np.ascontiguousarray(out[:, 1])
    return (out1, out2)


kernel.last_exec_time_ns = None
